# revision 20
# baseline (speedup 1.0000x reference)
"""Trainium2 Bass kernel for nn_CogAgentDecoderLayer (8-core SPMD).

Feature-major activations [feat, tok] in permuted token order
(vis-only | both | lang-only | neither). TP: QKV/self-attn by heads
(2/core), MLP by intermediate slice (688/core). Cross-attn K/V computed
sharded (128 of 1024 dims per core) + AllGathered early, overlapped with
self-attention. Self-attn ctx redistributed with AllToAll (head-shards ->
token-shards), then dense/cross-attn/cdense run token-parallel (256
tok/core). Final MLP partial sums reduced on host. Self-attention skips
fully-masked causal blocks; diagonal blocks use multiplicative 0/1 masks.
bf16 matmuls throughout, fp32 psum/residual/norm stats.
"""
import os
import numpy as np
from contextlib import ExitStack
from concourse import bacc, tile, mybir
from concourse.bass_utils import run_bass_kernel_spmd

NC_ = 8
S, E, H, NH, HD = 2048, 2048, 2048, 16, 128
CH, CC, CHD = 1024, 1024, 64
I = 5504
IS = I // NC_          # 688
ISP = 768              # padded to 6*128
EPS = 1e-5
ROPE_BASE = 10000.0
F32 = mybir.dt.float32
F32R = mybir.dt.float32r
BF16 = mybir.dt.bfloat16


def _segs(lo, hi, b0, b1, b2):
    pts = sorted({lo, hi, *[b for b in (b0, b1, b2) if lo < b < hi]})
    out = []
    for s, e in zip(pts, pts[1:]):
        ex = []
        if s < b1:
            ex.append(0)
        if b0 <= s < b2:
            ex.append(1)
        out.append((s, e, ex))
    return out


def _chunks(lo, hi, w):
    out = []
    while lo < hi:
        out.append((lo, min(lo + w, hi)))
        lo += w
    return out


def build_kernel(b0, b1, b2, blk, uniform):
    """blk: 64-tuple, state per (ci, kt): 0=skip, 1=visible, 2=partial.
    uniform: every 256-token chunk has a single expert-combo (host packs
    the right dense weight per core)."""
    nc = bacc.Bacc("TRN2", target_bir_lowering=False, debug=False,
                   num_devices=NC_)
    din = lambda n, sh, dt: nc.dram_tensor(n, sh, dt, kind="ExternalInput")
    hT = din("hT", [H, S], BF16)
    resid = din("resid", [H, 256], BF16)
    wqkv0 = din("wqkv0", [H, 768], BF16)
    wqkv1 = din("wqkv1", [H, 768], BF16)
    cos2 = din("cos2", [128, S], BF16)
    sin2 = din("sin2", [128, S], BF16)
    rotT = din("rotT", [128, 128], BF16)
    onesr = din("onesr", [128, 128], F32R)
    onesb = din("onesb", [128, 128], BF16)
    nmask = max(1, sum(1 for st in blk if st == 2))
    maskd = din("maskd", [128 * nmask, 512], BF16)
    encT = din("encT", [CH, E], BF16)
    wkc = din("wkc", [CH, 128], BF16)
    wvc = din("wvc", [CH, 128], BF16)
    if uniform:
        wde = din("wde", [H, H], BF16)
    else:
        wde0 = din("wde0", [H, H], BF16)
        wde1 = din("wde1", [H, H], BF16)
        mv_in = din("mv", [128, 256], BF16)
        ml_in = din("ml", [128, 256], BF16)
    wcq = din("wcq", [H, CC], BF16)
    wcd = din("wcd", [CC, H], BF16)
    wgu0 = din("wgu0", [H, 2 * IS], BF16)
    wgu1 = din("wgu1", [H, 2 * IS], BF16)
    wdn0 = din("wdn0", [ISP, H], BF16)
    wdn1 = din("wdn1", [ISP, H], BF16)
    y = nc.dram_tensor("y", [H, S], F32, kind="ExternalOutput")
    h2out = nc.dram_tensor("h2out", [H, 256], F32, kind="ExternalOutput")

    SC = 1.0 / float(np.sqrt(HD))
    CSC = 1.0 / float(np.sqrt(CHD))
    EXP = mybir.ActivationFunctionType.Exp
    SQ = mybir.ActivationFunctionType.Square
    SQRT = mybir.ActivationFunctionType.Sqrt
    SILU = mybir.ActivationFunctionType.Silu
    r128 = lambda ap: ap.rearrange("(c p) n -> p c n", p=128)
    RG = [list(range(NC_))]

    with tile.TileContext(nc) as tc, ExitStack() as top:
        const = top.enter_context(tc.tile_pool(name="const", bufs=1))
        ones_sb = const.tile([128, 128], F32R)
        nc.sync.dma_start(ones_sb[:], onesr.ap()[:])
        ones_bf = const.tile([128, 128], BF16)
        nc.sync.dma_start(ones_bf[:], onesb.ap()[:])
        rot_sb = const.tile([128, 128], BF16)
        nc.sync.dma_start(rot_sb[:], rotT.ap()[:])
        from concourse.masks import make_identity
        ident = const.tile([128, 128], BF16)
        make_identity(nc, ident[:])
        cos_sb = const.tile([128, S], BF16)
        nc.sync.dma_start(cos_sb[:], cos2.ap()[:])
        sin_sb = const.tile([128, S], BF16)
        nc.sync.dma_start(sin_sb[:], sin2.ap()[:])
        eps_sb = const.tile([128, 1], F32)
        nc.vector.memset(eps_sb[:], EPS)

        dram = top.enter_context(tc.tile_pool(name="dram", bufs=1, space="DRAM"))
        kbnc = dram.tile([128, E], BF16)
        vbnc = dram.tile([E, 128], BF16)
        kag = dram.tile([NC_ * 128, E], BF16)
        vag = dram.tile([NC_ * E, 128], BF16)
        a2ain = dram.tile([H, 256], BF16)
        a2aout = dram.tile([H, 256], BF16)
        h2nb = dram.tile([H, 256], BF16)
        h2na = dram.tile([NC_ * H, 256], BF16)

        scrp = top.enter_context(tc.tile_pool(name="scr", bufs=2))

        # h load issued first so it overlaps phase 0 compute
        pAB = top.enter_context(ExitStack())
        qkp = pAB.enter_context(tc.tile_pool(name="qkp", bufs=1))
        qkv_sb = qkp.tile([128, 6, S], BF16)      # q0 q1 k0 k1 v0 v1
        v_sb = qkp.tile([128, 16, 256], BF16)     # token-major v
        ctx_sb = qkp.tile([128, 2, S], BF16)
        hps = top.enter_context(ExitStack())
        hp = hps.enter_context(tc.tile_pool(name="hp", bufs=1))
        h_sb = hp.tile([128, 16, S], BF16)
        nc.sync.dma_start(h_sb[:], r128(hT.ap()))

        # ===== phase 0: cross K/V shard compute + AllGathers =====
        with ExitStack() as p0:
            ep = p0.enter_context(tc.tile_pool(name="ep", bufs=1))
            enc_sb = ep.tile([128, 8, E], BF16)
            nc.sync.dma_start(enc_sb[:], r128(encT.ap()))
            wk_sb = ep.tile([128, 8, 128], BF16)
            nc.sync.dma_start(wk_sb[:], r128(wkc.ap()))
            wv_sb = ep.tile([128, 8, 128], BF16)
            nc.sync.dma_start(wv_sb[:], r128(wvc.ap()))
            kb_sb = ep.tile([128, E], BF16)
            vb_sb = ep.tile([128, 16, 128], BF16)
            kvp = p0.enter_context(tc.tile_pool(name="kvp", bufs=2,
                                                space="PSUM"))
            for n0, n1 in _chunks(0, E, 512):
                ps = kvp.tile([128, 512], F32, name="kps", tag="kps")
                for kc in range(8):
                    nc.tensor.matmul(ps[:], wk_sb[:, kc, :],
                                     enc_sb[:, kc, n0:n1],
                                     start=(kc == 0), stop=(kc == 7))
                nc.vector.tensor_copy(kb_sb[:, n0:n1], ps[:])
            for kt in range(16):
                ps = kvp.tile([128, 128], F32, name="vps", tag="vps")
                for kc in range(8):
                    nc.tensor.matmul(ps[:],
                                     enc_sb[:, kc, kt * 128:kt * 128 + 128],
                                     wv_sb[:, kc, :],
                                     start=(kc == 0), stop=(kc == 7))
                nc.vector.tensor_copy(vb_sb[:, kt, :], ps[:])
            nc.sync.dma_start(kbnc[:], kb_sb[:])
            nc.sync.dma_start(vbnc[:].rearrange("(k p) d -> p k d", p=128),
                              vb_sb[:])
        nc.gpsimd.collective_compute(
            "AllGather", mybir.AluOpType.bypass, replica_groups=RG,
            ins=[kbnc.opt()], outs=[kag.opt()])
        nc.gpsimd.collective_compute(
            "AllGather", mybir.AluOpType.bypass, replica_groups=RG,
            ins=[vbnc.opt()], outs=[vag.opt()])

        # ===== phase A: rmsnorm + QKV + rope + vT =====
        with ExitStack() as pA1:
            nrm = pA1.enter_context(tc.tile_pool(name="nrm", bufs=2))
            nps = pA1.enter_context(tc.tile_pool(name="nps", bufs=2,
                                                 space="PSUM"))
            for t0, t1 in _chunks(0, S, 512):
                pss = nps.tile([128, 512], F32, name="pss", tag="pss")
                for kc in range(16):
                    sq = nrm.tile([128, 512], F32R, name="sq", tag="sq")
                    nc.scalar.activation(sq[:], h_sb[:, kc, t0:t1], SQ)
                    nc.tensor.matmul(pss[:], ones_sb[:], sq[:],
                                     start=(kc == 0), stop=(kc == 15))
                rms = nrm.tile([128, 512], F32, name="rms", tag="rms")
                nc.scalar.activation(rms[:], pss[:], SQRT,
                                     scale=1.0 / H, bias=eps_sb[:])
                rinv = nrm.tile([128, 512], F32, name="rinv", tag="rinv")
                nc.vector.reciprocal_approx_fast(rinv[:], rms[:])
                rinvb = nrm.tile([128, 512], BF16, name="rinvb", tag="rinvb")
                nc.vector.tensor_copy(rinvb[:], rinv[:])
                for kc in range(16):
                    nc.vector.tensor_mul(h_sb[:, kc, t0:t1],
                                         h_sb[:, kc, t0:t1], rinvb[:])
        with ExitStack() as pA2:
            wp = pA2.enter_context(tc.tile_pool(name="wp", bufs=3))
            mps = pA2.enter_context(tc.tile_pool(name="mps", bufs=2,
                                                 space="PSUM"))
            for slot in range(6):
                wts = []
                for ex, wsrc in ((0, wqkv0), (1, wqkv1)):
                    wt = wp.tile([128, 16, 128], BF16,
                                 name=f"wq{ex}{slot}", tag=f"wq{ex}")
                    nc.sync.dma_start(
                        wt[:], r128(wsrc.ap()[:, slot * 128:slot * 128 + 128]))
                    wts.append(wt)
                for t0, t1 in _chunks(0, S, 512):
                    sg = [x for x in _segs(t0, t1, b0, b1, b2) if x[2]]
                    if not sg:
                        continue
                    need = sorted({x for _, _, ex in sg for x in ex})
                    pss_ = {}
                    for x in need:
                        ps = mps.tile([128, 512], F32, name=f"qps{x}",
                                      tag=f"qps{x}")
                        for kc in range(16):
                            nc.tensor.matmul(ps[:], wts[x][:, kc, :],
                                             h_sb[:, kc, t0:t1],
                                             start=(kc == 0), stop=(kc == 15))
                        pss_[x] = ps
                    for s, e, ex in sg:
                        if len(ex) == 1:
                            nc.vector.tensor_copy(
                                qkv_sb[:, slot, s:e],
                                pss_[ex[0]][:, s - t0:e - t0])
                        else:
                            nc.vector.tensor_add(qkv_sb[:, slot, s:e],
                                                 pss_[0][:, s - t0:e - t0],
                                                 pss_[1][:, s - t0:e - t0])
                if b2 < S:
                    nc.vector.memset(qkv_sb[:, slot, b2:S], 0.0)
            # rope on q,k
            for slot in range(4):
                for t0, t1 in _chunks(0, S, 512):
                    rp = mps.tile([128, 512], F32, name="rps", tag="qps0")
                    nc.tensor.matmul(rp[:], rot_sb[:],
                                     qkv_sb[:, slot, t0:t1],
                                     start=True, stop=True)
                    rpb = scrp.tile([128, 512], BF16, name="rpb", tag="rpb")
                    nc.scalar.copy(rpb[:], rp[:])
                    c1 = scrp.tile([128, 512], BF16, name="ropec", tag="ropec")
                    nc.vector.tensor_mul(c1[:], qkv_sb[:, slot, t0:t1],
                                         cos_sb[:, t0:t1])
                    nc.vector.tensor_mul(rpb[:], rpb[:], sin_sb[:, t0:t1])
                    nc.vector.tensor_add(qkv_sb[:, slot, t0:t1],
                                         c1[:], rpb[:])
            # v -> token-major via PE transpose
            for hh in range(2):
                for tt in range(16):
                    tp = mps.tile([128, 128], BF16, name="tps", tag="tps")
                    nc.tensor.transpose(
                        tp[:],
                        qkv_sb[:, 4 + hh, tt * 128:tt * 128 + 128],
                        ident[:])
                    nc.vector.tensor_copy(
                        v_sb[:, tt, hh * 128:hh * 128 + 128], tp[:])
        hps.close()  # h_sb no longer needed; frees 8.4MB for prefetch

        # ===== phase B: self-attention (causal skip, mult. diag masks) =====
        mrow = {}
        _mr = 0
        for ci in range(4):
            for kt in range(16):
                if blk[ci * 16 + kt] == 2:
                    mrow[(ci, kt)] = _mr
                    _mr += 1
        with ExitStack() as pB:
            ap_ = pB.enter_context(tc.tile_pool(name="ap", bufs=3))
            aps = pB.enter_context(tc.tile_pool(name="aps", bufs=2,
                                                space="PSUM"))
            accp = pB.enter_context(tc.tile_pool(name="accp", bufs=1,
                                                 space="PSUM"))
            for ci, (t0, t1) in enumerate(_chunks(0, S, 512)):
                live = [(kt, blk[ci * 16 + kt]) for kt in range(16)
                        if blk[ci * 16 + kt] != 0]
                pss_ = [accp.tile([128, 512], F32, name=f"pbs{h}",
                                  tag=f"pbs{h}") for h in range(2)]
                psc_ = [accp.tile([128, 512], F32, name=f"pbc{h}",
                                  tag=f"pbc{h}") for h in range(2)]
                for li, (kt, st) in enumerate(live):
                    first, last = li == 0, li == len(live) - 1
                    if st == 2:
                        mt_ = ap_.tile([128, 512], BF16, name="mt", tag="mt")
                        r0 = mrow[(ci, kt)] * 128
                        nc.sync.dma_start(mt_[:],
                                          maskd.ap()[r0:r0 + 128, :])
                    for hh in range(2):
                        sc = aps.tile([128, 512], F32, name="sc", tag="sc")
                        nc.tensor.matmul(
                            sc[:], qkv_sb[:, 2 + hh, kt * 128:kt * 128 + 128],
                            qkv_sb[:, hh, t0:t1], start=True, stop=True)
                        pr = ap_.tile([128, 512], BF16, name="pr", tag="pr")
                        nc.scalar.activation(pr[:], sc[:], EXP, scale=SC)
                        if st == 2:
                            nc.vector.tensor_mul(pr[:], pr[:], mt_[:])
                        nc.tensor.matmul(pss_[hh][:], ones_bf[:], pr[:],
                                         start=first, stop=last)
                        nc.tensor.matmul(
                            psc_[hh][:],
                            v_sb[:, kt, hh * 128:hh * 128 + 128],
                            pr[:], start=first, stop=last)
                for hh in range(2):
                    rc = ap_.tile([128, 512], F32, name="rc", tag="rc")
                    nc.vector.reciprocal_approx_fast(rc[:], pss_[hh][:])
                    nc.vector.tensor_mul(ctx_sb[:, hh, t0:t1],
                                         psc_[hh][:], rc[:])
                for hh in range(2):
                    for j in (2 * ci, 2 * ci + 1):
                        nc.sync.dma_start(
                            a2ain[256 * j + 128 * hh:
                                  256 * j + 128 * hh + 128, :],
                            ctx_sb[:, hh, 256 * j:256 * j + 256])
        pAB.close()
        nc.gpsimd.collective_compute(
            "AllToAll", mybir.AluOpType.bypass, replica_groups=RG,
            ins=[a2ain.opt()], outs=[a2aout.opt()])

        # ===== phase C: dense (token-parallel) + h1 + rmsnorm + cq =====
        pCD = top.enter_context(ExitStack())
        cd = pCD.enter_context(tc.tile_pool(name="cd", bufs=1))
        h1_sb = cd.tile([128, 16, 256], F32)
        cq_sb = cd.tile([128, 8, 256], BF16)
        cctx_sb = cd.tile([128, 8, 256], BF16)
        dps = pCD.enter_context(tc.tile_pool(name="dps", bufs=2,
                                             space="PSUM"))
        kpre = pCD.enter_context(tc.tile_pool(name="kpre", bufs=1))
        k_sb = kpre.tile([128, 8, E], BF16)
        with ExitStack() as pC:
            cp = pC.enter_context(tc.tile_pool(name="cp", bufs=1))
            wdp = pC.enter_context(tc.tile_pool(name="wdp", bufs=2))
            cx_sb = cp.tile([128, 16, 256], BF16)
            nc.sync.dma_start(cx_sb[:], r128(a2aout[:]))
            re_sb = cp.tile([128, 16, 256], BF16)
            nc.sync.dma_start(re_sb[:], r128(resid.ap()))
            nc.sync.dma_start(k_sb[:], r128(kag[:]))  # prefetch for phase D
            wcq_sb = cp.tile([128, 16, CC], BF16)
            if not uniform:
                mv_sb = cp.tile([128, 256], BF16)
                nc.sync.dma_start(mv_sb[:], mv_in.ap()[:])
                ml_sb = cp.tile([128, 256], BF16)
                nc.sync.dma_start(ml_sb[:], ml_in.ap()[:])
                cxv = cp.tile([128, 16, 256], BF16)
                cxl = cp.tile([128, 16, 256], BF16)
                for kc in range(16):
                    nc.vector.tensor_mul(cxv[:, kc, :], cx_sb[:, kc, :],
                                         mv_sb[:])
                    nc.vector.tensor_mul(cxl[:, kc, :], cx_sb[:, kc, :],
                                         ml_sb[:])
            for mg in range(4):  # stream dense weight in 4 x 2.1MB tiles
                wsrc0 = wde if uniform else wde0
                wde_t = wdp.tile([128, 16, 512], BF16, name=f"wde{mg}",
                                 tag="wde")
                nc.sync.dma_start(
                    wde_t[:], r128(wsrc0.ap()[:, mg * 512:mg * 512 + 512]))
                if not uniform:
                    wde1_t = wdp.tile([128, 16, 512], BF16, name=f"wdeb{mg}",
                                      tag="wdeb")
                    nc.sync.dma_start(
                        wde1_t[:],
                        r128(wde1.ap()[:, mg * 512:mg * 512 + 512]))
                for mi in range(4):
                    mt = mg * 4 + mi
                    ps = dps.tile([128, 256], F32, name="dp", tag="dp")
                    for kc in range(16):
                        nc.tensor.matmul(
                            ps[:], wde_t[:, kc, mi * 128:mi * 128 + 128],
                            (cx_sb if uniform else cxv)[:, kc, :],
                            start=(kc == 0), stop=(kc == 15))
                    if uniform:
                        nc.vector.tensor_add(h1_sb[:, mt, :], ps[:],
                                             re_sb[:, mt, :])
                    else:
                        ps1 = dps.tile([128, 256], F32, name="dp1", tag="dp1")
                        for kc in range(16):
                            nc.tensor.matmul(
                                ps1[:],
                                wde1_t[:, kc, mi * 128:mi * 128 + 128],
                                cxl[:, kc, :],
                                start=(kc == 0), stop=(kc == 15))
                        t_ = scrp.tile([128, 256], F32, name="dt", tag="dt")
                        nc.vector.tensor_add(t_[:], ps[:], ps1[:])
                        nc.vector.tensor_add(h1_sb[:, mt, :], t_[:],
                                             re_sb[:, mt, :])
            nc.sync.dma_start(wcq_sb[:], r128(wcq.ap()))
            # rmsnorm h1 -> h1n (bf16)
            h1n_sb = cp.tile([128, 16, 256], BF16)
            pss = dps.tile([128, 256], F32, name="nps2", tag="dp")
            for kc in range(16):
                sq = scrp.tile([128, 256], F32R, name="sqc", tag="sqc")
                nc.scalar.activation(sq[:], h1_sb[:, kc, :], SQ)
                nc.tensor.matmul(pss[:], ones_sb[:], sq[:],
                                 start=(kc == 0), stop=(kc == 15))
            rms = scrp.tile([128, 256], F32, name="rmsc", tag="rmsc")
            nc.scalar.activation(rms[:], pss[:], SQRT,
                                 scale=1.0 / H, bias=eps_sb[:])
            rinv = scrp.tile([128, 256], F32, name="rinvc", tag="rmsc")
            nc.vector.reciprocal_approx_fast(rinv[:], rms[:])
            for kc in range(16):
                nc.vector.tensor_mul(h1n_sb[:, kc, :], h1_sb[:, kc, :],
                                     rinv[:])
            for mt in range(8):
                ps = dps.tile([128, 256], F32, name="cqp", tag="dp")
                for kc in range(16):
                    nc.tensor.matmul(ps[:],
                                     wcq_sb[:, kc, mt * 128:mt * 128 + 128],
                                     h1n_sb[:, kc, :],
                                     start=(kc == 0), stop=(kc == 15))
                nc.vector.tensor_copy(cq_sb[:, mt, :], ps[:])

        # ===== phase D: cross attention (token-parallel) + cdense =====
        with ExitStack() as pD:
            kp = pD.enter_context(tc.tile_pool(name="kp", bufs=1))
            v_sb2 = kp.tile([128, 16, CC], BF16)
            for r in range(NC_):
                nc.sync.dma_start(
                    v_sb2[:, :, r * 128:r * 128 + 128],
                    vag[r * E:(r + 1) * E, :].rearrange(
                        "(k p) d -> p k d", p=128))
            wcd_sb = kp.tile([128, 8, H], BF16)
            nc.sync.dma_start(wcd_sb[:], r128(wcd.ap()))
            cap = pD.enter_context(tc.tile_pool(name="cap", bufs=3))
            caps = pD.enter_context(tc.tile_pool(name="caps", bufs=2,
                                                 space="PSUM"))
            cacc = pD.enter_context(tc.tile_pool(name="cacc", bufs=1,
                                                 space="PSUM"))
            for m in range(8):  # head pair (2m, 2m+1)
                # psden slot 0 = head-a denominator, slot 1 = head-b
                psden = cacc.tile([128, 2, 256], F32, name="cps", tag="cps")
                pc2 = cacc.tile([128, 256], F32, name="cpc", tag="cpc")
                for kp in range(8):  # kt pair (2kp, 2kp+1)
                    kt0, kt1 = 2 * kp, 2 * kp + 1
                    # separate PSUM banks per head: concurrent row-group
                    # matmuls must not share a bank
                    sca = caps.tile([128, 2, 256], F32, name="csa", tag="csa")
                    scb = caps.tile([128, 2, 256], F32, name="csb", tag="csb")
                    for j, kt in ((0, kt0), (1, kt1)):
                        nc.tensor.matmul(
                            sca[:, j, :],
                            k_sb[0:64, m, kt * 128:kt * 128 + 128],
                            cq_sb[0:64, m, :], start=True, stop=True)
                        nc.tensor.matmul(
                            scb[:, j, :],
                            k_sb[64:128, m, kt * 128:kt * 128 + 128],
                            cq_sb[64:128, m, :], start=True, stop=True)
                    pra = cap.tile([128, 2, 256], BF16, name="cpra", tag="cpra")
                    nc.scalar.activation(pra[:], sca[:], EXP, scale=CSC)
                    prb = cap.tile([128, 2, 256], BF16, name="cprb", tag="cprb")
                    nc.scalar.activation(prb[:], scb[:], EXP, scale=CSC)
                    for j, kt in ((0, kt0), (1, kt1)):
                        nc.tensor.matmul(psden[:, 0, :], ones_bf[:],
                                         pra[:, j, :],
                                         start=(kp == 0 and j == 0),
                                         stop=(kp == 7 and j == 1))
                        nc.tensor.matmul(psden[:, 1, :], ones_bf[:],
                                         prb[:, j, :],
                                         start=(kp == 0 and j == 0),
                                         stop=(kp == 7 and j == 1))
                        nc.tensor.matmul(
                            pc2[0:64, :], v_sb2[:, kt, 128 * m:128 * m + 64],
                            pra[:, j, :], start=(kp == 0 and j == 0),
                            stop=(kp == 7 and j == 1))
                        nc.tensor.matmul(
                            pc2[64:128, :],
                            v_sb2[:, kt, 128 * m + 64:128 * m + 128],
                            prb[:, j, :], start=(kp == 0 and j == 0),
                            stop=(kp == 7 and j == 1))
                rc = cap.tile([64, 2, 256], F32, name="crc", tag="crc")
                nc.vector.reciprocal_approx_fast(rc[:], psden[0:64, :, :])
                nc.vector.tensor_mul(cctx_sb[0:64, m, :], pc2[0:64, :],
                                     rc[:, 0, :])
                nc.vector.tensor_mul(cctx_sb[64:128, m, :], pc2[64:128, :],
                                     rc[:, 1, :])
            # cdense + residual -> h2, rmsnorm -> h2n -> AG
            d4 = pD.enter_context(tc.tile_pool(name="d4", bufs=1))
            h2_sb = d4.tile([128, 16, 256], F32)
            h2n_sb = d4.tile([128, 16, 256], BF16)
            for mt in range(16):
                ps = dps.tile([128, 256], F32, name="cdp", tag="dp")
                for kc in range(8):
                    nc.tensor.matmul(ps[:],
                                     wcd_sb[:, kc, mt * 128:mt * 128 + 128],
                                     cctx_sb[:, kc, :],
                                     start=(kc == 0), stop=(kc == 7))
                nc.vector.tensor_add(h2_sb[:, mt, :], ps[:],
                                     h1_sb[:, mt, :])
            pss2 = dps.tile([128, 256], F32, name="psd2", tag="dp")
            for kc in range(16):
                sq = scrp.tile([128, 256], F32R, name="sqd2", tag="sqc")
                nc.scalar.activation(sq[:], h2_sb[:, kc, :], SQ)
                nc.tensor.matmul(pss2[:], ones_sb[:], sq[:],
                                 start=(kc == 0), stop=(kc == 15))
            rms2 = scrp.tile([128, 256], F32, name="rmsd2", tag="rmsc")
            nc.scalar.activation(rms2[:], pss2[:], SQRT,
                                 scale=1.0 / H, bias=eps_sb[:])
            rinv2 = scrp.tile([128, 256], F32, name="rinvd", tag="rmsc")
            nc.vector.reciprocal_approx_fast(rinv2[:], rms2[:])
            for kc in range(16):
                nc.vector.tensor_mul(h2n_sb[:, kc, :],
                                     h2_sb[:, kc, :], rinv2[:])
            nc.sync.dma_start(r128(h2nb[:]), h2n_sb[:])
            nc.sync.dma_start(r128(h2out.ap()), h2_sb[:])
        pCD.close()
        nc.gpsimd.collective_compute(
            "AllGather", mybir.AluOpType.bypass, replica_groups=RG,
            ins=[h2nb.opt()], outs=[h2na.opt()])

        # ===== phase F: MLP (routed by expert ranges, bf16) =====
        with ExitStack() as pF:
            fp = pF.enter_context(tc.tile_pool(name="fp", bufs=1))
            hn_sb = fp.tile([128, 16, S], BF16)
            for r in range(NC_):
                nc.sync.dma_start(hn_sb[:, :, r * 256:r * 256 + 256],
                                  r128(h2na[r * H:(r + 1) * H, :]))
            fw = pF.enter_context(tc.tile_pool(name="fw", bufs=1))
            fps = pF.enter_context(tc.tile_pool(name="fps", bufs=2,
                                                space="PSUM"))
            fpd = pF.enter_context(tc.tile_pool(name="fpd", bufs=2,
                                                space="PSUM"))
            fac = pF.enter_context(tc.tile_pool(name="fac", bufs=2))
            fout = pF.enter_context(tc.tile_pool(name="fout", bufs=4))
            for ex, (lo, hi) in ((0, (0, b1)), (1, (b1, S))):
                gsrc = (wgu0, wgu1)[ex]
                dsrc = (wdn0, wdn1)[ex]
                dn_t = fw.tile([128, 6, H], BF16, name=f"dn{ex}", tag="dn")
                nc.sync.dma_start(dn_t[:], r128(dsrc.ap()))
                gwts = []
                for pi in range(6):
                    gw = 128 if pi < 5 else 48
                    gwt = fw.tile([128, 16, 256], BF16,
                                  name=f"guw{ex}{pi}", tag=f"guw{pi}")
                    nc.sync.dma_start(
                        gwt[:, :, :2 * gw],
                        r128(gsrc.ap()[:, pi * 256:pi * 256 + 2 * gw]))
                    gwts.append(gwt)
                for a0 in range(0, S, 512):
                    c0, c1 = max(a0, lo), min(a0 + 512, hi)
                    if c0 >= c1:
                        continue
                    t0_, W = a0, 512
                    eo, ew = c0 - a0, c1 - c0
                    act = fac.tile([128, 6, 512], BF16, name="act", tag="act")
                    for pi in range(6):
                        gw = 128 if pi < 5 else 48
                        gwt = gwts[pi]
                        pg = fps.tile([128, 512], F32, name="pg", tag="pg")
                        pu = fps.tile([128, 512], F32, name="pu", tag="pu")
                        for kc in range(16):
                            nc.tensor.matmul(pg[:gw, :W], gwt[:, kc, :gw],
                                             hn_sb[:, kc, t0_:t0_ + 512],
                                             start=(kc == 0), stop=(kc == 15))
                            nc.tensor.matmul(pu[:gw, :W], gwt[:, kc, gw:2 * gw],
                                             hn_sb[:, kc, t0_:t0_ + 512],
                                             start=(kc == 0), stop=(kc == 15))
                        gs = scrp.tile([128, 512], F32, name="gs", tag="gs")
                        nc.scalar.activation(gs[:gw, :W], pg[:gw, :W], SILU)
                        nc.vector.tensor_mul(act[:gw, pi, :W],
                                             gs[:gw, :W], pu[:gw, :W])
                    for mt in range(16):
                        pd = fpd.tile([128, 512], F32, name="pd", tag="pd")
                        for pi in range(6):
                            kw = 128 if pi < 5 else 48
                            nc.tensor.matmul(
                                pd[:, :W],
                                dn_t[:kw, pi, mt * 128:mt * 128 + 128],
                                act[:kw, pi, :W],
                                start=(pi == 0), stop=(pi == 5))
                        ot = fout.tile([128, 512], F32, name="fot", tag="fot")
                        if mt % 2 == 0:
                            nc.vector.tensor_copy(ot[:, eo:eo + ew],
                                                  pd[:, eo:eo + ew])
                        else:
                            nc.scalar.copy(ot[:, eo:eo + ew],
                                           pd[:, eo:eo + ew])
                        nc.sync.dma_start(
                            y.ap()[mt * 128:mt * 128 + 128, c0:c1],
                            ot[:, eo:eo + ew])
    nc.compile()
    return nc


_CACHE = {}


def kernel(**inputs):
    import ml_dtypes
    vm = np.asarray(inputs["vision_token_ids"]).astype(bool)
    lm = np.asarray(inputs["language_token_ids"]).astype(bool)
    g0 = np.where(vm & ~lm)[0]; g1 = np.where(vm & lm)[0]
    g2 = np.where(~vm & lm)[0]; g3 = np.where(~vm & ~lm)[0]
    perm = np.concatenate([g0, g1, g2, g3])
    b0 = len(g0); b1 = b0 + len(g1); b2 = b1 + len(g2)

    f32 = lambda x: np.ascontiguousarray(np.asarray(x, np.float32))
    bf = lambda x: np.ascontiguousarray(np.asarray(x).astype(ml_dtypes.bfloat16))
    pos = np.asarray(inputs["positions"]).astype(np.float32)
    half = HD // 2
    inv_freq = 1.0 / (ROPE_BASE ** (np.arange(half, dtype=np.float32) / half))
    fr = pos[:, None] * inv_freq[None, :]
    cos2 = np.concatenate([np.cos(fr)] * 2, 1).T[:, perm]
    sin2 = np.concatenate([np.sin(fr)] * 2, 1).T[:, perm]
    rot = np.zeros((HD, HD), np.float32)
    rot[np.arange(half), np.arange(half) + half] = -1.0
    rot[np.arange(half) + half, np.arange(half)] = 1.0
    op = np.asarray(inputs["positions"])[perm]

    # causal block states + diagonal multiplicative masks
    blk = []
    mrows = []
    for ci in range(4):
        qv = op[512 * ci:512 * ci + 512]
        for kt in range(16):
            kv = op[128 * kt:128 * kt + 128]
            if kv.max() <= qv.min():
                blk.append(1)
            elif kv.min() > qv.max():
                blk.append(0)
            else:
                blk.append(2)
                mrows.append((qv[None, :] >= kv[:, None]).astype(np.float32))
    blk = tuple(blk)
    maskd = (np.concatenate(mrows, 0) if mrows
             else np.zeros((128, 512), np.float32))

    # per-chunk expert combos (0=vis, 1=both, 2=lang, 3=neither)
    combo = np.full(S, 3, np.int8)
    combo[:b0] = 0; combo[b0:b1] = 1; combo[b1:b2] = 2
    chunk_combo = []
    uniform = True
    for j in range(NC_):
        cj = combo[256 * j:256 * j + 256]
        if (cj == cj[0]).all():
            chunk_combo.append(int(cj[0]))
        else:
            chunk_combo.append(-1)
            uniform = False

    wln_in = f32(inputs["w_ln_in"])[:, None]
    wln_pa = f32(inputs["w_ln_post_attn"])[:, None]
    wln_pc = f32(inputs["w_ln_post_cross"])[:, None]
    wqkv = [f32(inputs["w_vis_qkv"]) * wln_in, f32(inputs["w_lang_qkv"]) * wln_in]
    wd = [f32(inputs["w_vis_dense"]), f32(inputs["w_lang_dense"])]
    wgu = [f32(inputs["w_vis_gate_up"]) * wln_pc,
           f32(inputs["w_lang_gate_up"]) * wln_pc]
    wdn = [f32(inputs["w_vis_down"]), f32(inputs["w_lang_down"])]
    wkvf = f32(inputs["w_cross_kv"])
    hTp = f32(inputs["hidden_states"]).T[:, perm].copy()

    def interleave(w):  # w [H, 2*IS] = [gate | up]
        cols = []
        for i in range(5):
            cols.append(w[:, 128 * i:128 * i + 128])
            cols.append(w[:, IS + 128 * i:IS + 128 * i + 128])
        cols.append(w[:, 640:IS]); cols.append(w[:, IS + 640:2 * IS])
        return np.ascontiguousarray(np.concatenate(cols, 1))

    key = (b0, b1, b2, blk, uniform)
    if key not in _CACHE:
        _CACHE.clear()
        _CACHE[key] = build_kernel(b0, b1, b2, blk, uniform)
    nc = _CACHE[key]

    # dense weight combos (bf16, built once per distinct combo)
    wde_by_combo = {}
    for cb in set(chunk_combo):
        if cb == 0:
            wde_by_combo[cb] = bf(wd[0])
        elif cb == 1:
            wde_by_combo[cb] = bf(wd[0] + wd[1])
        elif cb == 2:
            wde_by_combo[cb] = bf(wd[1])
        elif cb == 3:
            wde_by_combo[cb] = bf(np.zeros_like(wd[0]))

    in_maps = []
    for c in range(NC_):
        qs = slice(256 * c, 256 * c + 256)
        m = dict(
            hT=bf(hTp),
            resid=bf(hTp[:, qs]),
            wqkv0=bf(np.concatenate([wqkv[0][:, qs], wqkv[0][:, H:][:, qs],
                                     wqkv[0][:, 2 * H:][:, qs]], 1)),
            wqkv1=bf(np.concatenate([wqkv[1][:, qs], wqkv[1][:, H:][:, qs],
                                     wqkv[1][:, 2 * H:][:, qs]], 1)),
            cos2=bf(cos2), sin2=bf(sin2), rotT=bf(rot.T),
            onesr=np.ones((128, 128), np.float32),
            onesb=np.ones((128, 128), ml_dtypes.bfloat16),
            maskd=bf(maskd),
            encT=bf(f32(inputs["encoder_embeds"]).T),
            wkc=bf(wkvf[:, 128 * c:128 * c + 128]),
            wvc=bf(wkvf[:, CC + 128 * c:CC + 128 * c + 128]),
            wcq=bf(f32(inputs["w_cross_q"]) * wln_pa),
            wcd=bf(f32(inputs["w_cross_dense"])),
            wgu0=bf(interleave(np.concatenate(
                [wgu[0][:, IS * c:IS * c + IS],
                 wgu[0][:, I + IS * c:I + IS * c + IS]], 1))),
            wgu1=bf(interleave(np.concatenate(
                [wgu[1][:, IS * c:IS * c + IS],
                 wgu[1][:, I + IS * c:I + IS * c + IS]], 1))),
            wdn0=bf(np.concatenate([wdn[0][IS * c:IS * c + IS],
                                    np.zeros((ISP - IS, H), np.float32)], 0)),
            wdn1=bf(np.concatenate([wdn[1][IS * c:IS * c + IS],
                                    np.zeros((ISP - IS, H), np.float32)], 0)),
        )
        if uniform:
            m["wde"] = wde_by_combo[chunk_combo[c]]
        else:
            m["wde0"] = bf(wd[0])
            m["wde1"] = bf(wd[1])
            pv = vm[perm][qs].astype(np.float32)
            pl = lm[perm][qs].astype(np.float32)
            m["mv"] = bf(np.broadcast_to(pv[None, :], (128, 256)).copy())
            m["ml"] = bf(np.broadcast_to(pl[None, :], (128, 256)).copy())
        in_maps.append(m)

    # wqkv slot layout check: slots are [q(2x128) | k(2x128) | v(2x128)]
    # per-core head pair -> columns 128c..128c+256 of each of q,k,v.

    if os.environ.get("KSIM"):
        from concourse.bass_interp import MultiCoreSim
        sim = MultiCoreSim(nc, num_cores=NC_)
        for c, cs in sim.cores.items():
            for name, val in in_maps[c].items():
                cs.tensor(name)[:] = val
        sim.simulate(check_with_hw=False)
        results = [dict(y=np.array(sim.cores[c].tensor("y")),
                        h2out=np.array(sim.cores[c].tensor("h2out")))
                   for c in range(NC_)]
        kernel.last_exec_ns = 0
    else:
        trace = bool(int(os.environ.get("KTRACE", "0")))
        res = run_bass_kernel_spmd(nc, in_maps, core_ids=list(range(NC_)),
                                   trace=trace)
        kernel.last_exec_ns = res.exec_time_ns
        results = res.results
    tot = results[0]["y"].astype(np.float64)
    for c in range(1, NC_):
        tot += results[c]["y"]
    for c in range(NC_):
        tot[:, 256 * c:256 * c + 256] += results[c]["h2out"]
    out = np.empty((S, H), np.float32)
    out[perm, :] = tot.T.astype(np.float32)
    return out


# revision 23
# speedup vs baseline: 1.1051x; 1.1051x over previous
"""Trainium2 Bass kernel for nn_CogAgentDecoderLayer (8-core SPMD).

Feature-major activations [feat, tok] in permuted token order
(vis-only | both | lang-only | neither). TP: QKV/self-attn by heads
(2/core), MLP by intermediate slice (688/core). Cross-attn K/V computed
sharded (128 of 1024 dims per core) + AllGathered early, overlapped with
self-attention. Self-attn ctx redistributed with AllToAll (head-shards ->
token-shards), then dense/cross-attn/cdense run token-parallel (256
tok/core). Final MLP partial sums reduced on host. Self-attention skips
fully-masked causal blocks; diagonal blocks use multiplicative 0/1 masks.
bf16 matmuls throughout, fp32 psum/residual/norm stats.
"""
import os
import numpy as np
from contextlib import ExitStack
from concourse import bacc, tile, mybir
from concourse.bass_utils import run_bass_kernel_spmd

NC_ = 8
S, E, H, NH, HD = 2048, 2048, 2048, 16, 128
CH, CC, CHD = 1024, 1024, 64
I = 5504
IS = I // NC_          # 688
ISP = 768              # padded to 6*128
EPS = 1e-5
ROPE_BASE = 10000.0
F32 = mybir.dt.float32
F32R = mybir.dt.float32r
BF16 = mybir.dt.bfloat16


def _segs(lo, hi, b0, b1, b2):
    pts = sorted({lo, hi, *[b for b in (b0, b1, b2) if lo < b < hi]})
    out = []
    for s, e in zip(pts, pts[1:]):
        ex = []
        if s < b1:
            ex.append(0)
        if b0 <= s < b2:
            ex.append(1)
        out.append((s, e, ex))
    return out


def _chunks(lo, hi, w):
    out = []
    while lo < hi:
        out.append((lo, min(lo + w, hi)))
        lo += w
    return out


def build_kernel(b0, b1, b2, blk, uniform):
    """blk: 64-tuple, state per (ci, kt): 0=skip, 1=visible, 2=partial.
    uniform: every 256-token chunk has a single expert-combo (host packs
    the right dense weight per core)."""
    nc = bacc.Bacc("TRN2", target_bir_lowering=False, debug=False,
                   num_devices=NC_)
    din = lambda n, sh, dt: nc.dram_tensor(n, sh, dt, kind="ExternalInput")
    hT = din("hT", [H, S], BF16)
    resid = din("resid", [H, 256], BF16)
    wqkv0 = din("wqkv0", [H, 768], BF16)
    wqkv1 = din("wqkv1", [H, 768], BF16)
    cos2 = din("cos2", [128, S], BF16)
    sin2 = din("sin2", [128, S], BF16)
    rotT = din("rotT", [128, 128], BF16)
    onesb = din("onesb", [128, 128], BF16)
    nmask = max(1, sum(1 for st in blk if st == 2))
    maskd = din("maskd", [128 * nmask, 512], BF16)
    encT = din("encT", [CH, E], BF16)
    wkc = din("wkc", [CH, 128], BF16)
    wvc = din("wvc", [CH, 128], BF16)
    if uniform:
        wde = din("wde", [H, H], BF16)
    else:
        wde0 = din("wde0", [H, H], BF16)
        wde1 = din("wde1", [H, H], BF16)
        mv_in = din("mv", [128, 256], BF16)
        ml_in = din("ml", [128, 256], BF16)
    wcq = din("wcq", [H, CC], BF16)
    wcd = din("wcd", [CC, H], BF16)
    wgu0 = din("wgu0", [H, 2 * IS], BF16)
    wgu1 = din("wgu1", [H, 2 * IS], BF16)
    wdn0 = din("wdn0", [ISP, H], BF16)
    wdn1 = din("wdn1", [ISP, H], BF16)
    y = nc.dram_tensor("y", [H, S], F32, kind="ExternalOutput")
    h2out = nc.dram_tensor("h2out", [H, 256], F32, kind="ExternalOutput")

    SC = 1.0 / float(np.sqrt(HD))
    CSC = 1.0 / float(np.sqrt(CHD))
    EXP = mybir.ActivationFunctionType.Exp
    SQ = mybir.ActivationFunctionType.Square
    SQRT = mybir.ActivationFunctionType.Sqrt
    SILU = mybir.ActivationFunctionType.Silu
    r128 = lambda ap: ap.rearrange("(c p) n -> p c n", p=128)
    RG = [list(range(NC_))]

    with tile.TileContext(nc) as tc, ExitStack() as top:
        const = top.enter_context(tc.tile_pool(name="const", bufs=1))
        ones_bf = const.tile([128, 128], BF16)
        nc.sync.dma_start(ones_bf[:], onesb.ap()[:])
        rot_sb = const.tile([128, 128], BF16)
        nc.sync.dma_start(rot_sb[:], rotT.ap()[:])
        from concourse.masks import make_identity
        ident = const.tile([128, 128], BF16)
        make_identity(nc, ident[:])
        cos_sb = const.tile([128, S], BF16)
        nc.sync.dma_start(cos_sb[:], cos2.ap()[:])
        sin_sb = const.tile([128, S], BF16)
        nc.sync.dma_start(sin_sb[:], sin2.ap()[:])
        eps_sb = const.tile([128, 1], F32)
        nc.vector.memset(eps_sb[:], EPS)

        dram = top.enter_context(tc.tile_pool(name="dram", bufs=1, space="DRAM"))
        kbnc = dram.tile([128, E], BF16)
        vbnc = dram.tile([E, 128], BF16)
        kag = dram.tile([NC_ * 128, E], BF16, addr_space="Shared")
        vag = dram.tile([NC_ * E, 128], BF16, addr_space="Shared")
        a2ain = dram.tile([H, 256], BF16)
        a2aout = dram.tile([H, 256], BF16)
        h2nb = dram.tile([H, 256], BF16)
        h2na = dram.tile([NC_ * H, 256], BF16, addr_space="Shared")

        scrp = top.enter_context(tc.tile_pool(name="scr", bufs=2))

        # h load issued first so it overlaps phase 0 compute
        pAB = top.enter_context(ExitStack())
        qkp = pAB.enter_context(tc.tile_pool(name="qkp", bufs=1))
        qkv_sb = qkp.tile([128, 6, S], BF16)      # q0 q1 k0 k1 v0 v1
        v_sb = qkp.tile([128, 16, 256], BF16)     # token-major v
        ctx_sb = qkp.tile([128, 2, S], BF16)
        hps = top.enter_context(ExitStack())
        hp = hps.enter_context(tc.tile_pool(name="hp", bufs=1))
        h_sb = hp.tile([128, 16, S], BF16)
        nc.sync.dma_start(h_sb[:], r128(hT.ap()))

        # ===== phase 0: cross K/V shard compute + AllGathers =====
        with ExitStack() as p0:
            ep = p0.enter_context(tc.tile_pool(name="ep", bufs=1))
            enc_sb = ep.tile([128, 8, E], BF16)
            nc.sync.dma_start(enc_sb[:], r128(encT.ap()))
            wk_sb = ep.tile([128, 8, 128], BF16)
            nc.sync.dma_start(wk_sb[:], r128(wkc.ap()))
            wv_sb = ep.tile([128, 8, 128], BF16)
            nc.sync.dma_start(wv_sb[:], r128(wvc.ap()))
            kb_sb = ep.tile([128, E], BF16)
            vb_sb = ep.tile([128, 16, 128], BF16)
            kvp = p0.enter_context(tc.tile_pool(name="kvp", bufs=2,
                                                space="PSUM"))
            for n0, n1 in _chunks(0, E, 512):
                ps = kvp.tile([128, 512], F32, name="kps", tag="kps")
                for kc in range(8):
                    nc.tensor.matmul(ps[:], wk_sb[:, kc, :],
                                     enc_sb[:, kc, n0:n1],
                                     start=(kc == 0), stop=(kc == 7))
                nc.vector.tensor_copy(kb_sb[:, n0:n1], ps[:])
            for kt in range(16):
                ps = kvp.tile([128, 128], F32, name="vps", tag="vps")
                for kc in range(8):
                    nc.tensor.matmul(ps[:],
                                     enc_sb[:, kc, kt * 128:kt * 128 + 128],
                                     wv_sb[:, kc, :],
                                     start=(kc == 0), stop=(kc == 7))
                nc.vector.tensor_copy(vb_sb[:, kt, :], ps[:])
            nc.sync.dma_start(kbnc[:], kb_sb[:])
            nc.sync.dma_start(vbnc[:].rearrange("(k p) d -> p k d", p=128),
                              vb_sb[:])
        nc.gpsimd.collective_compute(
            "AllGather", mybir.AluOpType.bypass, replica_groups=RG,
            ins=[kbnc.opt()], outs=[kag.opt()])
        nc.gpsimd.collective_compute(
            "AllGather", mybir.AluOpType.bypass, replica_groups=RG,
            ins=[vbnc.opt()], outs=[vag.opt()])

        # ===== phase A: per-chunk rmsnorm + QKV + rope + vT =====
        with ExitStack() as pA2:
            wp = pA2.enter_context(tc.tile_pool(name="wp", bufs=1))
            wq_sb = [wp.tile([128, 16, 768], BF16, name=f"wqa{x}",
                             tag=f"wqa{x}") for x in range(2)]
            nc.sync.dma_start(wq_sb[0][:], r128(wqkv0.ap()))
            nc.sync.dma_start(wq_sb[1][:], r128(wqkv1.ap()))
            nrm = pA2.enter_context(tc.tile_pool(name="nrm", bufs=2))
            mps = pA2.enter_context(tc.tile_pool(name="mps", bufs=2,
                                                 space="PSUM"))
            for ci, (t0, t1) in enumerate(_chunks(0, S, 512)):
                pss = mps.tile([128, 512], F32, name="pss", tag="qps1")
                for kc in range(16):
                    sq = nrm.tile([128, 512], BF16, name="sq", tag="sq")
                    nc.scalar.activation(sq[:], h_sb[:, kc, t0:t1], SQ)
                    nc.tensor.matmul(pss[:], ones_bf[:], sq[:],
                                     start=(kc == 0), stop=(kc == 15))
                rms = nrm.tile([128, 512], F32, name="rms", tag="rms")
                nc.scalar.activation(rms[:], pss[:], SQRT,
                                     scale=1.0 / H, bias=eps_sb[:])
                rinv = nrm.tile([128, 512], F32, name="rinv", tag="rinv")
                nc.vector.reciprocal_approx_fast(rinv[:], rms[:])
                rinvb = nrm.tile([128, 512], BF16, name="rinvb", tag="rinvb")
                nc.vector.tensor_copy(rinvb[:], rinv[:])
                for kc in range(16):
                    nc.vector.tensor_mul(h_sb[:, kc, t0:t1],
                                         h_sb[:, kc, t0:t1], rinvb[:])
                sg = [x for x in _segs(t0, t1, b0, b1, b2) if x[2]]
                for slot in range(6):
                    if sg:
                        need = sorted({x for _, _, ex in sg for x in ex})
                        pss_ = {}
                        for x in need:
                            ps = mps.tile([128, 512], F32, name=f"qps{x}",
                                          tag=f"qps{x}")
                            for kc in range(16):
                                nc.tensor.matmul(
                                    ps[:],
                                    wq_sb[x][:, kc, slot * 128:slot * 128 + 128],
                                    h_sb[:, kc, t0:t1],
                                    start=(kc == 0), stop=(kc == 15))
                            pss_[x] = ps
                        for s, e, ex in sg:
                            if len(ex) == 1:
                                nc.vector.tensor_copy(
                                    qkv_sb[:, slot, s:e],
                                    pss_[ex[0]][:, s - t0:e - t0])
                            else:
                                nc.vector.tensor_add(qkv_sb[:, slot, s:e],
                                                     pss_[0][:, s - t0:e - t0],
                                                     pss_[1][:, s - t0:e - t0])
                    if t1 > b2 > t0:
                        nc.vector.memset(qkv_sb[:, slot, b2:t1], 0.0)
                    elif t0 >= b2:
                        nc.vector.memset(qkv_sb[:, slot, t0:t1], 0.0)
                # rope on q,k of this chunk
                for slot in range(4):
                    rp = mps.tile([128, 512], F32, name="rps", tag="qps0")
                    nc.tensor.matmul(rp[:], rot_sb[:],
                                     qkv_sb[:, slot, t0:t1],
                                     start=True, stop=True)
                    rpb = scrp.tile([128, 512], BF16, name="rpb", tag="rpb")
                    nc.scalar.copy(rpb[:], rp[:])
                    c1 = scrp.tile([128, 512], BF16, name="ropec", tag="ropec")
                    nc.vector.tensor_mul(c1[:], qkv_sb[:, slot, t0:t1],
                                         cos_sb[:, t0:t1])
                    nc.vector.tensor_mul(rpb[:], rpb[:], sin_sb[:, t0:t1])
                    nc.vector.tensor_add(qkv_sb[:, slot, t0:t1],
                                         c1[:], rpb[:])
                # v -> token-major via PE transpose (this chunk's tokens)
                for hh in range(2):
                    for tt in range(t0 // 128, t1 // 128):
                        tp = mps.tile([128, 128], BF16, name="tps", tag="tps")
                        nc.tensor.transpose(
                            tp[:],
                            qkv_sb[:, 4 + hh, tt * 128:tt * 128 + 128],
                            ident[:])
                        nc.vector.tensor_copy(
                            v_sb[:, tt, hh * 128:hh * 128 + 128], tp[:])
        hps.close()  # h_sb no longer needed; frees 8.4MB for prefetch

        # ===== phase B: self-attention (causal skip, mult. diag masks) =====
        mrow = {}
        _mr = 0
        for ci in range(4):
            for kt in range(16):
                if blk[ci * 16 + kt] == 2:
                    mrow[(ci, kt)] = _mr
                    _mr += 1
        with ExitStack() as pB:
            ap_ = pB.enter_context(tc.tile_pool(name="ap", bufs=3))
            aps = pB.enter_context(tc.tile_pool(name="aps", bufs=2,
                                                space="PSUM"))
            accp = pB.enter_context(tc.tile_pool(name="accp", bufs=1,
                                                 space="PSUM"))
            for ci, (t0, t1) in enumerate(_chunks(0, S, 512)):
                live = [(kt, blk[ci * 16 + kt]) for kt in range(16)
                        if blk[ci * 16 + kt] != 0]
                pss_ = [accp.tile([128, 512], F32, name=f"pbs{h}",
                                  tag=f"pbs{h}") for h in range(2)]
                psc_ = [accp.tile([128, 512], F32, name=f"pbc{h}",
                                  tag=f"pbc{h}") for h in range(2)]
                for li, (kt, st) in enumerate(live):
                    first, last = li == 0, li == len(live) - 1
                    if st == 2:
                        mt_ = ap_.tile([128, 512], BF16, name="mt", tag="mt")
                        r0 = mrow[(ci, kt)] * 128
                        nc.sync.dma_start(mt_[:],
                                          maskd.ap()[r0:r0 + 128, :])
                    for hh in range(2):
                        sc = aps.tile([128, 512], F32, name="sc", tag="sc")
                        nc.tensor.matmul(
                            sc[:], qkv_sb[:, 2 + hh, kt * 128:kt * 128 + 128],
                            qkv_sb[:, hh, t0:t1], start=True, stop=True)
                        pr = ap_.tile([128, 512], BF16, name="pr", tag="pr")
                        nc.scalar.activation(pr[:], sc[:], EXP, scale=SC)
                        if st == 2:
                            nc.vector.tensor_mul(pr[:], pr[:], mt_[:])
                        nc.tensor.matmul(pss_[hh][:], ones_bf[:], pr[:],
                                         start=first, stop=last)
                        nc.tensor.matmul(
                            psc_[hh][:],
                            v_sb[:, kt, hh * 128:hh * 128 + 128],
                            pr[:], start=first, stop=last)
                for hh in range(2):
                    rc = ap_.tile([128, 512], F32, name="rc", tag="rc")
                    nc.vector.reciprocal_approx_fast(rc[:], pss_[hh][:])
                    nc.vector.tensor_mul(ctx_sb[:, hh, t0:t1],
                                         psc_[hh][:], rc[:])
                for hh in range(2):
                    for j in (2 * ci, 2 * ci + 1):
                        nc.sync.dma_start(
                            a2ain[256 * j + 128 * hh:
                                  256 * j + 128 * hh + 128, :],
                            ctx_sb[:, hh, 256 * j:256 * j + 256])
        pAB.close()
        nc.gpsimd.collective_compute(
            "AllToAll", mybir.AluOpType.bypass, replica_groups=RG,
            ins=[a2ain.opt()], outs=[a2aout.opt()])

        # ===== phase C: dense (token-parallel) + h1 + rmsnorm + cq =====
        pCD = top.enter_context(ExitStack())
        cd = pCD.enter_context(tc.tile_pool(name="cd", bufs=1))
        h1_sb = cd.tile([128, 16, 256], F32)
        cq_sb = cd.tile([128, 8, 256], BF16)
        cctx_sb = cd.tile([128, 8, 256], BF16)
        dps = pCD.enter_context(tc.tile_pool(name="dps", bufs=2,
                                             space="PSUM"))
        kpre = pCD.enter_context(tc.tile_pool(name="kpre", bufs=1))
        k_sb = kpre.tile([128, 8, E], BF16)
        with ExitStack() as pC:
            cp = pC.enter_context(tc.tile_pool(name="cp", bufs=1))
            wdp = pC.enter_context(tc.tile_pool(name="wdp", bufs=2))
            cx_sb = cp.tile([128, 16, 256], BF16)
            nc.sync.dma_start(cx_sb[:], r128(a2aout[:]))
            re_sb = cp.tile([128, 16, 256], BF16)
            nc.sync.dma_start(re_sb[:], r128(resid.ap()))
            nc.sync.dma_start(k_sb[:], r128(kag[:]))  # prefetch for phase D
            wcq_sb = cp.tile([128, 16, CC], BF16)
            if not uniform:
                mv_sb = cp.tile([128, 256], BF16)
                nc.sync.dma_start(mv_sb[:], mv_in.ap()[:])
                ml_sb = cp.tile([128, 256], BF16)
                nc.sync.dma_start(ml_sb[:], ml_in.ap()[:])
                cxv = cp.tile([128, 16, 256], BF16)
                cxl = cp.tile([128, 16, 256], BF16)
                for kc in range(16):
                    nc.vector.tensor_mul(cxv[:, kc, :], cx_sb[:, kc, :],
                                         mv_sb[:])
                    nc.vector.tensor_mul(cxl[:, kc, :], cx_sb[:, kc, :],
                                         ml_sb[:])
            for mg in range(4):  # stream dense weight in 4 x 2.1MB tiles
                wsrc0 = wde if uniform else wde0
                wde_t = wdp.tile([128, 16, 512], BF16, name=f"wde{mg}",
                                 tag="wde")
                nc.sync.dma_start(
                    wde_t[:], r128(wsrc0.ap()[:, mg * 512:mg * 512 + 512]))
                if not uniform:
                    wde1_t = wdp.tile([128, 16, 512], BF16, name=f"wdeb{mg}",
                                      tag="wdeb")
                    nc.sync.dma_start(
                        wde1_t[:],
                        r128(wde1.ap()[:, mg * 512:mg * 512 + 512]))
                for mi in range(4):
                    mt = mg * 4 + mi
                    ps = dps.tile([128, 256], F32, name="dp", tag="dp")
                    for kc in range(16):
                        nc.tensor.matmul(
                            ps[:], wde_t[:, kc, mi * 128:mi * 128 + 128],
                            (cx_sb if uniform else cxv)[:, kc, :],
                            start=(kc == 0), stop=(kc == 15))
                    if uniform:
                        nc.vector.tensor_add(h1_sb[:, mt, :], ps[:],
                                             re_sb[:, mt, :])
                    else:
                        ps1 = dps.tile([128, 256], F32, name="dp1", tag="dp1")
                        for kc in range(16):
                            nc.tensor.matmul(
                                ps1[:],
                                wde1_t[:, kc, mi * 128:mi * 128 + 128],
                                cxl[:, kc, :],
                                start=(kc == 0), stop=(kc == 15))
                        t_ = scrp.tile([128, 256], F32, name="dt", tag="dt")
                        nc.vector.tensor_add(t_[:], ps[:], ps1[:])
                        nc.vector.tensor_add(h1_sb[:, mt, :], t_[:],
                                             re_sb[:, mt, :])
            nc.sync.dma_start(wcq_sb[:], r128(wcq.ap()))
            # rmsnorm h1 -> h1n (bf16)
            h1n_sb = cp.tile([128, 16, 256], BF16)
            pss = dps.tile([128, 256], F32, name="nps2", tag="dp")
            for kc in range(16):
                sq = scrp.tile([128, 256], BF16, name="sqc", tag="sqc")
                nc.scalar.activation(sq[:], h1_sb[:, kc, :], SQ)
                nc.tensor.matmul(pss[:], ones_bf[:], sq[:],
                                 start=(kc == 0), stop=(kc == 15))
            rms = scrp.tile([128, 256], F32, name="rmsc", tag="rmsc")
            nc.scalar.activation(rms[:], pss[:], SQRT,
                                 scale=1.0 / H, bias=eps_sb[:])
            rinv = scrp.tile([128, 256], F32, name="rinvc", tag="rmsc")
            nc.vector.reciprocal_approx_fast(rinv[:], rms[:])
            for kc in range(16):
                nc.vector.tensor_mul(h1n_sb[:, kc, :], h1_sb[:, kc, :],
                                     rinv[:])
            for mt in range(8):
                ps = dps.tile([128, 256], F32, name="cqp", tag="dp")
                for kc in range(16):
                    nc.tensor.matmul(ps[:],
                                     wcq_sb[:, kc, mt * 128:mt * 128 + 128],
                                     h1n_sb[:, kc, :],
                                     start=(kc == 0), stop=(kc == 15))
                nc.vector.tensor_copy(cq_sb[:, mt, :], ps[:])

        # ===== phase D: cross attention (token-parallel) + cdense =====
        with ExitStack() as pD:
            kp = pD.enter_context(tc.tile_pool(name="kp", bufs=1))
            v_sb2 = kp.tile([128, 16, CC], BF16)
            for r in range(NC_):
                nc.sync.dma_start(
                    v_sb2[:, :, r * 128:r * 128 + 128],
                    vag[r * E:(r + 1) * E, :].rearrange(
                        "(k p) d -> p k d", p=128))
            wcd_sb = kp.tile([128, 8, H], BF16)
            nc.sync.dma_start(wcd_sb[:], r128(wcd.ap()))
            cap = pD.enter_context(tc.tile_pool(name="cap", bufs=3))
            caps = pD.enter_context(tc.tile_pool(name="caps", bufs=2,
                                                 space="PSUM"))
            cacc = pD.enter_context(tc.tile_pool(name="cacc", bufs=1,
                                                 space="PSUM"))
            for m in range(8):  # head pair (2m, 2m+1)
                # psden slot 0 = head-a denominator, slot 1 = head-b
                psden = cacc.tile([128, 2, 256], F32, name="cps", tag="cps")
                pc2 = cacc.tile([128, 256], F32, name="cpc", tag="cpc")
                for kp in range(8):  # kt pair (2kp, 2kp+1)
                    kt0, kt1 = 2 * kp, 2 * kp + 1
                    # separate PSUM banks per head: concurrent row-group
                    # matmuls must not share a bank
                    sca = caps.tile([128, 2, 256], F32, name="csa", tag="csa")
                    scb = caps.tile([128, 2, 256], F32, name="csb", tag="csb")
                    for j, kt in ((0, kt0), (1, kt1)):
                        nc.tensor.matmul(
                            sca[:, j, :],
                            k_sb[0:64, m, kt * 128:kt * 128 + 128],
                            cq_sb[0:64, m, :], start=True, stop=True)
                        nc.tensor.matmul(
                            scb[:, j, :],
                            k_sb[64:128, m, kt * 128:kt * 128 + 128],
                            cq_sb[64:128, m, :], start=True, stop=True)
                    pra = cap.tile([128, 2, 256], BF16, name="cpra", tag="cpra")
                    nc.scalar.activation(pra[:], sca[:], EXP, scale=CSC)
                    prb = cap.tile([128, 2, 256], BF16, name="cprb", tag="cprb")
                    nc.scalar.activation(prb[:], scb[:], EXP, scale=CSC)
                    for j, kt in ((0, kt0), (1, kt1)):
                        nc.tensor.matmul(psden[:, 0, :], ones_bf[:],
                                         pra[:, j, :],
                                         start=(kp == 0 and j == 0),
                                         stop=(kp == 7 and j == 1))
                        nc.tensor.matmul(psden[:, 1, :], ones_bf[:],
                                         prb[:, j, :],
                                         start=(kp == 0 and j == 0),
                                         stop=(kp == 7 and j == 1))
                        nc.tensor.matmul(
                            pc2[0:64, :], v_sb2[:, kt, 128 * m:128 * m + 64],
                            pra[:, j, :], start=(kp == 0 and j == 0),
                            stop=(kp == 7 and j == 1))
                        nc.tensor.matmul(
                            pc2[64:128, :],
                            v_sb2[:, kt, 128 * m + 64:128 * m + 128],
                            prb[:, j, :], start=(kp == 0 and j == 0),
                            stop=(kp == 7 and j == 1))
                rc = cap.tile([64, 2, 256], F32, name="crc", tag="crc")
                nc.vector.reciprocal_approx_fast(rc[:], psden[0:64, :, :])
                nc.vector.tensor_mul(cctx_sb[0:64, m, :], pc2[0:64, :],
                                     rc[:, 0, :])
                nc.vector.tensor_mul(cctx_sb[64:128, m, :], pc2[64:128, :],
                                     rc[:, 1, :])
            # cdense + residual -> h2, rmsnorm -> h2n -> AG
            d4 = pD.enter_context(tc.tile_pool(name="d4", bufs=1))
            h2_sb = d4.tile([128, 16, 256], F32)
            h2n_sb = d4.tile([128, 16, 256], BF16)
            for mt in range(16):
                ps = dps.tile([128, 256], F32, name="cdp", tag="dp")
                for kc in range(8):
                    nc.tensor.matmul(ps[:],
                                     wcd_sb[:, kc, mt * 128:mt * 128 + 128],
                                     cctx_sb[:, kc, :],
                                     start=(kc == 0), stop=(kc == 7))
                nc.vector.tensor_add(h2_sb[:, mt, :], ps[:],
                                     h1_sb[:, mt, :])
            pss2 = dps.tile([128, 256], F32, name="psd2", tag="dp")
            for kc in range(16):
                sq = scrp.tile([128, 256], BF16, name="sqd2", tag="sqc")
                nc.scalar.activation(sq[:], h2_sb[:, kc, :], SQ)
                nc.tensor.matmul(pss2[:], ones_bf[:], sq[:],
                                 start=(kc == 0), stop=(kc == 15))
            rms2 = scrp.tile([128, 256], F32, name="rmsd2", tag="rmsc")
            nc.scalar.activation(rms2[:], pss2[:], SQRT,
                                 scale=1.0 / H, bias=eps_sb[:])
            rinv2 = scrp.tile([128, 256], F32, name="rinvd", tag="rmsc")
            nc.vector.reciprocal_approx_fast(rinv2[:], rms2[:])
            for kc in range(16):
                nc.vector.tensor_mul(h2n_sb[:, kc, :],
                                     h2_sb[:, kc, :], rinv2[:])
            nc.sync.dma_start(r128(h2nb[:]), h2n_sb[:])
            nc.sync.dma_start(r128(h2out.ap()), h2_sb[:])
        pCD.close()
        nc.gpsimd.collective_compute(
            "AllGather", mybir.AluOpType.bypass, replica_groups=RG,
            ins=[h2nb.opt()], outs=[h2na.opt()])

        # ===== phase F: MLP (routed by expert ranges, bf16) =====
        with ExitStack() as pF:
            fp = pF.enter_context(tc.tile_pool(name="fp", bufs=1))
            hn_sb = fp.tile([128, 16, S], BF16)
            for r in range(NC_):
                nc.sync.dma_start(hn_sb[:, :, r * 256:r * 256 + 256],
                                  r128(h2na[r * H:(r + 1) * H, :]))
            fw = pF.enter_context(tc.tile_pool(name="fw", bufs=1))
            fps = pF.enter_context(tc.tile_pool(name="fps", bufs=2,
                                                space="PSUM"))
            fpd = pF.enter_context(tc.tile_pool(name="fpd", bufs=2,
                                                space="PSUM"))
            fac = pF.enter_context(tc.tile_pool(name="fac", bufs=2))
            fout = pF.enter_context(tc.tile_pool(name="fout", bufs=4))
            for ex, (lo, hi) in ((0, (0, b1)), (1, (b1, S))):
                gsrc = (wgu0, wgu1)[ex]
                dsrc = (wdn0, wdn1)[ex]
                dn_t = fw.tile([128, 6, H], BF16, name=f"dn{ex}", tag="dn")
                nc.sync.dma_start(dn_t[:], r128(dsrc.ap()))
                gwts = []
                for pi in range(6):
                    gw = 128 if pi < 5 else 48
                    gwt = fw.tile([128, 16, 256], BF16,
                                  name=f"guw{ex}{pi}", tag=f"guw{pi}")
                    nc.sync.dma_start(
                        gwt[:, :, :2 * gw],
                        r128(gsrc.ap()[:, pi * 256:pi * 256 + 2 * gw]))
                    gwts.append(gwt)
                for a0 in range(0, S, 512):
                    c0, c1 = max(a0, lo), min(a0 + 512, hi)
                    if c0 >= c1:
                        continue
                    t0_, W = a0, 512
                    eo, ew = c0 - a0, c1 - c0
                    act = fac.tile([128, 6, 512], BF16, name="act", tag="act")
                    for pi in range(6):
                        gw = 128 if pi < 5 else 48
                        gwt = gwts[pi]
                        pg = fps.tile([128, 512], F32, name="pg", tag="pg")
                        pu = fps.tile([128, 512], F32, name="pu", tag="pu")
                        for kc in range(16):
                            nc.tensor.matmul(pg[:gw, :W], gwt[:, kc, :gw],
                                             hn_sb[:, kc, t0_:t0_ + 512],
                                             start=(kc == 0), stop=(kc == 15))
                            nc.tensor.matmul(pu[:gw, :W], gwt[:, kc, gw:2 * gw],
                                             hn_sb[:, kc, t0_:t0_ + 512],
                                             start=(kc == 0), stop=(kc == 15))
                        gs = scrp.tile([128, 512], F32, name="gs", tag="gs")
                        nc.scalar.activation(gs[:gw, :W], pg[:gw, :W], SILU)
                        nc.vector.tensor_mul(act[:gw, pi, :W],
                                             gs[:gw, :W], pu[:gw, :W])
                    for mt in range(16):
                        pd = fpd.tile([128, 512], F32, name="pd", tag="pd")
                        for pi in range(6):
                            kw = 128 if pi < 5 else 48
                            nc.tensor.matmul(
                                pd[:, :W],
                                dn_t[:kw, pi, mt * 128:mt * 128 + 128],
                                act[:kw, pi, :W],
                                start=(pi == 0), stop=(pi == 5))
                        ot = fout.tile([128, 512], F32, name="fot", tag="fot")
                        if mt % 2 == 0:
                            nc.vector.tensor_copy(ot[:, eo:eo + ew],
                                                  pd[:, eo:eo + ew])
                        else:
                            nc.scalar.copy(ot[:, eo:eo + ew],
                                           pd[:, eo:eo + ew])
                        nc.sync.dma_start(
                            y.ap()[mt * 128:mt * 128 + 128, c0:c1],
                            ot[:, eo:eo + ew])
    nc.compile()
    return nc


_CACHE = {}


def kernel(**inputs):
    import ml_dtypes
    vm = np.asarray(inputs["vision_token_ids"]).astype(bool)
    lm = np.asarray(inputs["language_token_ids"]).astype(bool)
    g0 = np.where(vm & ~lm)[0]; g1 = np.where(vm & lm)[0]
    g2 = np.where(~vm & lm)[0]; g3 = np.where(~vm & ~lm)[0]
    perm = np.concatenate([g0, g1, g2, g3])
    b0 = len(g0); b1 = b0 + len(g1); b2 = b1 + len(g2)

    f32 = lambda x: np.ascontiguousarray(np.asarray(x, np.float32))
    bf = lambda x: np.ascontiguousarray(np.asarray(x).astype(ml_dtypes.bfloat16))
    pos = np.asarray(inputs["positions"]).astype(np.float32)
    half = HD // 2
    inv_freq = 1.0 / (ROPE_BASE ** (np.arange(half, dtype=np.float32) / half))
    fr = pos[:, None] * inv_freq[None, :]
    cos2 = np.concatenate([np.cos(fr)] * 2, 1).T[:, perm]
    sin2 = np.concatenate([np.sin(fr)] * 2, 1).T[:, perm]
    rot = np.zeros((HD, HD), np.float32)
    rot[np.arange(half), np.arange(half) + half] = -1.0
    rot[np.arange(half) + half, np.arange(half)] = 1.0
    op = np.asarray(inputs["positions"])[perm]

    # causal block states + diagonal multiplicative masks
    blk = []
    mrows = []
    for ci in range(4):
        qv = op[512 * ci:512 * ci + 512]
        for kt in range(16):
            kv = op[128 * kt:128 * kt + 128]
            if kv.max() <= qv.min():
                blk.append(1)
            elif kv.min() > qv.max():
                blk.append(0)
            else:
                blk.append(2)
                mrows.append((qv[None, :] >= kv[:, None]).astype(np.float32))
    blk = tuple(blk)
    maskd = (np.concatenate(mrows, 0) if mrows
             else np.zeros((128, 512), np.float32))

    # per-chunk expert combos (0=vis, 1=both, 2=lang, 3=neither)
    combo = np.full(S, 3, np.int8)
    combo[:b0] = 0; combo[b0:b1] = 1; combo[b1:b2] = 2
    chunk_combo = []
    uniform = True
    for j in range(NC_):
        cj = combo[256 * j:256 * j + 256]
        if (cj == cj[0]).all():
            chunk_combo.append(int(cj[0]))
        else:
            chunk_combo.append(-1)
            uniform = False

    wln_in = f32(inputs["w_ln_in"])[:, None]
    wln_pa = f32(inputs["w_ln_post_attn"])[:, None]
    wln_pc = f32(inputs["w_ln_post_cross"])[:, None]
    wqkv = [f32(inputs["w_vis_qkv"]) * wln_in, f32(inputs["w_lang_qkv"]) * wln_in]
    wd = [f32(inputs["w_vis_dense"]), f32(inputs["w_lang_dense"])]
    wgu = [f32(inputs["w_vis_gate_up"]) * wln_pc,
           f32(inputs["w_lang_gate_up"]) * wln_pc]
    wdn = [f32(inputs["w_vis_down"]), f32(inputs["w_lang_down"])]
    wkvf = f32(inputs["w_cross_kv"])
    hTp = f32(inputs["hidden_states"]).T[:, perm].copy()

    def interleave(w):  # w [H, 2*IS] = [gate | up]
        cols = []
        for i in range(5):
            cols.append(w[:, 128 * i:128 * i + 128])
            cols.append(w[:, IS + 128 * i:IS + 128 * i + 128])
        cols.append(w[:, 640:IS]); cols.append(w[:, IS + 640:2 * IS])
        return np.ascontiguousarray(np.concatenate(cols, 1))

    key = (b0, b1, b2, blk, uniform)
    if key not in _CACHE:
        _CACHE.clear()
        _CACHE[key] = build_kernel(b0, b1, b2, blk, uniform)
    nc = _CACHE[key]

    # dense weight combos (bf16, built once per distinct combo)
    wde_by_combo = {}
    for cb in set(chunk_combo):
        if cb == 0:
            wde_by_combo[cb] = bf(wd[0])
        elif cb == 1:
            wde_by_combo[cb] = bf(wd[0] + wd[1])
        elif cb == 2:
            wde_by_combo[cb] = bf(wd[1])
        elif cb == 3:
            wde_by_combo[cb] = bf(np.zeros_like(wd[0]))

    in_maps = []
    for c in range(NC_):
        qs = slice(256 * c, 256 * c + 256)
        m = dict(
            hT=bf(hTp),
            resid=bf(hTp[:, qs]),
            wqkv0=bf(np.concatenate([wqkv[0][:, qs], wqkv[0][:, H:][:, qs],
                                     wqkv[0][:, 2 * H:][:, qs]], 1)),
            wqkv1=bf(np.concatenate([wqkv[1][:, qs], wqkv[1][:, H:][:, qs],
                                     wqkv[1][:, 2 * H:][:, qs]], 1)),
            cos2=bf(cos2), sin2=bf(sin2), rotT=bf(rot.T),
            onesb=np.ones((128, 128), ml_dtypes.bfloat16),
            maskd=bf(maskd),
            encT=bf(f32(inputs["encoder_embeds"]).T),
            wkc=bf(wkvf[:, 128 * c:128 * c + 128]),
            wvc=bf(wkvf[:, CC + 128 * c:CC + 128 * c + 128]),
            wcq=bf(f32(inputs["w_cross_q"]) * wln_pa),
            wcd=bf(f32(inputs["w_cross_dense"])),
            wgu0=bf(interleave(np.concatenate(
                [wgu[0][:, IS * c:IS * c + IS],
                 wgu[0][:, I + IS * c:I + IS * c + IS]], 1))),
            wgu1=bf(interleave(np.concatenate(
                [wgu[1][:, IS * c:IS * c + IS],
                 wgu[1][:, I + IS * c:I + IS * c + IS]], 1))),
            wdn0=bf(np.concatenate([wdn[0][IS * c:IS * c + IS],
                                    np.zeros((ISP - IS, H), np.float32)], 0)),
            wdn1=bf(np.concatenate([wdn[1][IS * c:IS * c + IS],
                                    np.zeros((ISP - IS, H), np.float32)], 0)),
        )
        if uniform:
            m["wde"] = wde_by_combo[chunk_combo[c]]
        else:
            m["wde0"] = bf(wd[0])
            m["wde1"] = bf(wd[1])
            pv = vm[perm][qs].astype(np.float32)
            pl = lm[perm][qs].astype(np.float32)
            m["mv"] = bf(np.broadcast_to(pv[None, :], (128, 256)).copy())
            m["ml"] = bf(np.broadcast_to(pl[None, :], (128, 256)).copy())
        in_maps.append(m)

    # wqkv slot layout check: slots are [q(2x128) | k(2x128) | v(2x128)]
    # per-core head pair -> columns 128c..128c+256 of each of q,k,v.

    if os.environ.get("KSIM"):
        from concourse.bass_interp import MultiCoreSim
        sim = MultiCoreSim(nc, num_cores=NC_)
        for c, cs in sim.cores.items():
            for name, val in in_maps[c].items():
                cs.tensor(name)[:] = val
        sim.simulate(check_with_hw=False)
        results = [dict(y=np.array(sim.cores[c].tensor("y")),
                        h2out=np.array(sim.cores[c].tensor("h2out")))
                   for c in range(NC_)]
        kernel.last_exec_ns = 0
    else:
        trace = bool(int(os.environ.get("KTRACE", "0")))
        res = run_bass_kernel_spmd(nc, in_maps, core_ids=list(range(NC_)),
                                   trace=trace)
        kernel.last_exec_ns = res.exec_time_ns
        results = res.results
    tot = results[0]["y"].astype(np.float64)
    for c in range(1, NC_):
        tot += results[c]["y"]
    for c in range(NC_):
        tot[:, 256 * c:256 * c + 256] += results[c]["h2out"]
    out = np.empty((S, H), np.float32)
    out[perm, :] = tot.T.astype(np.float32)
    return out


# revision 28
# speedup vs baseline: 1.1914x; 1.0780x over previous
"""Trainium2 Bass kernel for nn_CogAgentDecoderLayer (8-core SPMD).

Feature-major activations [feat, tok] in permuted token order
(vis-only | both | lang-only | neither). TP: QKV/self-attn by heads
(2/core), MLP by intermediate slice (688/core). Cross-attn K/V computed
sharded (128 of 1024 dims per core) + AllGathered early, overlapped with
self-attention. Self-attn ctx redistributed with AllToAll (head-shards ->
token-shards), then dense/cross-attn/cdense run token-parallel (256
tok/core). Final MLP partial sums reduced on host. Self-attention skips
fully-masked causal blocks; diagonal blocks use multiplicative 0/1 masks.
bf16 matmuls throughout, fp32 psum/residual/norm stats.
"""
import os
import numpy as np
from contextlib import ExitStack
from concourse import bacc, tile, mybir
from concourse.bass_utils import run_bass_kernel_spmd

NC_ = 8
S, E, H, NH, HD = 2048, 2048, 2048, 16, 128
CH, CC, CHD = 1024, 1024, 64
I = 5504
IS = I // NC_          # 688
ISP = 768              # padded to 6*128
EPS = 1e-5
ROPE_BASE = 10000.0
F32 = mybir.dt.float32
F32R = mybir.dt.float32r
BF16 = mybir.dt.bfloat16


def _segs(lo, hi, b0, b1, b2):
    pts = sorted({lo, hi, *[b for b in (b0, b1, b2) if lo < b < hi]})
    out = []
    for s, e in zip(pts, pts[1:]):
        ex = []
        if s < b1:
            ex.append(0)
        if b0 <= s < b2:
            ex.append(1)
        out.append((s, e, ex))
    return out


def _chunks(lo, hi, w):
    out = []
    while lo < hi:
        out.append((lo, min(lo + w, hi)))
        lo += w
    return out


def build_kernel(b0, b1, b2, blk, uniform):
    """blk: 64-tuple, state per (ci, kt): 0=skip, 1=visible, 2=partial.
    uniform: every 256-token chunk has a single expert-combo (host packs
    the right dense weight per core)."""
    nc = bacc.Bacc("TRN2", target_bir_lowering=False, debug=False,
                   num_devices=NC_)
    din = lambda n, sh, dt: nc.dram_tensor(n, sh, dt, kind="ExternalInput")
    hT = din("hT", [H, S], BF16)
    resid = din("resid", [H, 256], BF16)
    wqkv0 = din("wqkv0", [H, 768], BF16)
    wqkv1 = din("wqkv1", [H, 768], BF16)
    cos2 = din("cos2", [128, S], BF16)
    sin2 = din("sin2", [128, S], BF16)
    rotT = din("rotT", [128, 128], BF16)
    onesb = din("onesb", [128, 128], BF16)
    nmask = max(1, sum(1 for st in blk if st == 2))
    maskd = din("maskd", [128 * nmask, 512], BF16)
    encT = din("encT", [CH, E], BF16)
    wkc = din("wkc", [CH, 128], BF16)
    wvc = din("wvc", [CH, 128], BF16)
    if uniform:
        wde = din("wde", [H, H], BF16)
    else:
        wde0 = din("wde0", [H, H], BF16)
        wde1 = din("wde1", [H, H], BF16)
        mv_in = din("mv", [128, 256], BF16)
        ml_in = din("ml", [128, 256], BF16)
    wcq = din("wcq", [H, CC], BF16)
    wcd = din("wcd", [CC, H], BF16)
    wgu0 = din("wgu0", [H, 2 * IS], BF16)
    wgu1 = din("wgu1", [H, 2 * IS], BF16)
    wdn0 = din("wdn0", [ISP, H], BF16)
    wdn1 = din("wdn1", [ISP, H], BF16)
    y = nc.dram_tensor("y", [H, S], F32, kind="ExternalOutput")
    h2out = nc.dram_tensor("h2out", [H, 256], F32, kind="ExternalOutput")

    SC = 1.0 / float(np.sqrt(HD))
    CSC = 1.0 / float(np.sqrt(CHD))
    EXP = mybir.ActivationFunctionType.Exp
    SQ = mybir.ActivationFunctionType.Square
    SQRT = mybir.ActivationFunctionType.Sqrt
    SILU = mybir.ActivationFunctionType.Silu
    r128 = lambda ap: ap.rearrange("(c p) n -> p c n", p=128)
    RG = [list(range(NC_))]

    with tile.TileContext(nc) as tc, ExitStack() as top:
        const = top.enter_context(tc.tile_pool(name="const", bufs=1))
        ones_bf = const.tile([128, 128], BF16)
        nc.sync.dma_start(ones_bf[:], onesb.ap()[:])
        rot_sb = const.tile([128, 128], BF16)
        nc.sync.dma_start(rot_sb[:], rotT.ap()[:])
        from concourse.masks import make_identity
        ident = const.tile([128, 128], BF16)
        make_identity(nc, ident[:])
        cos_sb = const.tile([128, S], BF16)
        sin_sb = const.tile([128, S], BF16)
        eps_sb = const.tile([128, 1], F32)
        nc.vector.memset(eps_sb[:], EPS)

        dram = top.enter_context(tc.tile_pool(name="dram", bufs=1, space="DRAM"))
        kbnc = dram.tile([128, E], BF16)
        vbnc = dram.tile([E, 128], BF16)
        kag = dram.tile([NC_ * 128, E], BF16, addr_space="Shared")
        vag = dram.tile([NC_ * E, 128], BF16, addr_space="Shared")
        a2ain = dram.tile([H, 256], BF16)
        a2aout = dram.tile([H, 256], BF16)
        h2nb = dram.tile([H, 256], BF16)
        h2na = dram.tile([NC_ * H, 256], BF16, addr_space="Shared")

        scrp = top.enter_context(tc.tile_pool(name="scr", bufs=2))

        # tiles for A/B; h DMA deferred until after phase-0 inputs so the
        # cross-KV compute (first PE work) isn't starved by the h transfer
        pAB = top.enter_context(ExitStack())
        qkp = pAB.enter_context(tc.tile_pool(name="qkp", bufs=1))
        qkv_sb = qkp.tile([128, 6, S], BF16)      # q0 q1 k0 k1 v0 v1
        v_sb = qkp.tile([128, 16, 256], BF16)     # token-major v
        ctx_sb = qkp.tile([128, 2, S], BF16)
        hps = top.enter_context(ExitStack())
        hp = hps.enter_context(tc.tile_pool(name="hp", bufs=1))
        h_sb = hp.tile([128, 16, S], BF16)

        # ===== phase 0: cross K/V shard compute + AllGathers =====
        with ExitStack() as p0:
            ep = p0.enter_context(tc.tile_pool(name="ep", bufs=1))
            enc_sb = ep.tile([128, 8, E], BF16)
            nc.sync.dma_start(enc_sb[:], r128(encT.ap()))
            wk_sb = ep.tile([128, 8, 128], BF16)
            nc.sync.dma_start(wk_sb[:], r128(wkc.ap()))
            wv_sb = ep.tile([128, 8, 128], BF16)
            nc.sync.dma_start(wv_sb[:], r128(wvc.ap()))
            kb_sb = ep.tile([128, E], BF16)
            vb_sb = ep.tile([128, 16, 128], BF16)
            kvp = p0.enter_context(tc.tile_pool(name="kvp", bufs=2,
                                                space="PSUM"))
            for n0, n1 in _chunks(0, E, 512):
                ps = kvp.tile([128, 512], F32, name="kps", tag="kps")
                for kc in range(8):
                    nc.tensor.matmul(ps[:], wk_sb[:, kc, :],
                                     enc_sb[:, kc, n0:n1],
                                     start=(kc == 0), stop=(kc == 7))
                nc.vector.tensor_copy(kb_sb[:, n0:n1], ps[:])
            for kt in range(16):
                ps = kvp.tile([128, 128], F32, name="vps", tag="vps")
                for kc in range(8):
                    nc.tensor.matmul(ps[:],
                                     enc_sb[:, kc, kt * 128:kt * 128 + 128],
                                     wv_sb[:, kc, :],
                                     start=(kc == 0), stop=(kc == 7))
                nc.vector.tensor_copy(vb_sb[:, kt, :], ps[:])
            nc.sync.dma_start(kbnc[:], kb_sb[:])
            nc.sync.dma_start(vbnc[:].rearrange("(k p) d -> p k d", p=128),
                              vb_sb[:])
        nc.gpsimd.collective_compute(
            "AllGather", mybir.AluOpType.bypass, replica_groups=RG,
            ins=[kbnc.opt()], outs=[kag.opt()])
        nc.gpsimd.collective_compute(
            "AllGather", mybir.AluOpType.bypass, replica_groups=RG,
            ins=[vbnc.opt()], outs=[vag.opt()])

        # ===== phase A: per-chunk rmsnorm + QKV + rope + vT =====
        with ExitStack() as pA2:
            wp = pA2.enter_context(tc.tile_pool(name="wp", bufs=1))
            wq_sb = [wp.tile([128, 16, 768], BF16, name=f"wqa{x}",
                             tag=f"wqa{x}") for x in range(2)]
            # DMA order tuned for earliest first matmul: h chunk 0, then
            # QKV weights, then the rest of h, then rope tables
            nc.sync.dma_start(
                h_sb[:, :, 0:512],
                hT.ap()[:, 0:512].rearrange("(c p) n -> p c n", p=128))
            nc.sync.dma_start(wq_sb[0][:], r128(wqkv0.ap()))
            nc.sync.dma_start(wq_sb[1][:], r128(wqkv1.ap()))
            for t0, t1 in _chunks(512, S, 512):
                nc.sync.dma_start(
                    h_sb[:, :, t0:t1],
                    hT.ap()[:, t0:t1].rearrange("(c p) n -> p c n", p=128))
            nc.sync.dma_start(cos_sb[:], cos2.ap()[:])
            nc.sync.dma_start(sin_sb[:], sin2.ap()[:])
            nrm = pA2.enter_context(tc.tile_pool(name="nrm", bufs=2))
            mps = pA2.enter_context(tc.tile_pool(name="mps", bufs=2,
                                                 space="PSUM"))
            for ci, (t0, t1) in enumerate(_chunks(0, S, 512)):
                pss = mps.tile([128, 512], F32, name="pss", tag="qps1")
                for kc in range(16):
                    sq = nrm.tile([128, 512], BF16, name="sq", tag="sq")
                    nc.scalar.activation(sq[:], h_sb[:, kc, t0:t1], SQ)
                    nc.tensor.matmul(pss[:], ones_bf[:], sq[:],
                                     start=(kc == 0), stop=(kc == 15))
                rms = nrm.tile([128, 512], F32, name="rms", tag="rms")
                nc.scalar.activation(rms[:], pss[:], SQRT,
                                     scale=1.0 / H, bias=eps_sb[:])
                rinv = nrm.tile([128, 512], F32, name="rinv", tag="rinv")
                nc.vector.reciprocal_approx_fast(rinv[:], rms[:])
                rinvb = nrm.tile([128, 512], BF16, name="rinvb", tag="rinvb")
                nc.vector.tensor_copy(rinvb[:], rinv[:])
                for kc in range(16):
                    nc.vector.tensor_mul(h_sb[:, kc, t0:t1],
                                         h_sb[:, kc, t0:t1], rinvb[:])
                sg = [x for x in _segs(t0, t1, b0, b1, b2) if x[2]]
                for slot in range(6):
                    if sg:
                        need = sorted({x for _, _, ex in sg for x in ex})
                        pss_ = {}
                        for x in need:
                            ps = mps.tile([128, 512], F32, name=f"qps{x}",
                                          tag=f"qps{x}")
                            for kc in range(16):
                                nc.tensor.matmul(
                                    ps[:],
                                    wq_sb[x][:, kc, slot * 128:slot * 128 + 128],
                                    h_sb[:, kc, t0:t1],
                                    start=(kc == 0), stop=(kc == 15))
                            pss_[x] = ps
                        for s, e, ex in sg:
                            if len(ex) == 1:
                                nc.vector.tensor_copy(
                                    qkv_sb[:, slot, s:e],
                                    pss_[ex[0]][:, s - t0:e - t0])
                            else:
                                nc.vector.tensor_add(qkv_sb[:, slot, s:e],
                                                     pss_[0][:, s - t0:e - t0],
                                                     pss_[1][:, s - t0:e - t0])
                    if t1 > b2 > t0:
                        nc.vector.memset(qkv_sb[:, slot, b2:t1], 0.0)
                    elif t0 >= b2:
                        nc.vector.memset(qkv_sb[:, slot, t0:t1], 0.0)
                # rope on q,k of this chunk
                for slot in range(4):
                    rp = mps.tile([128, 512], F32, name="rps", tag="qps0")
                    nc.tensor.matmul(rp[:], rot_sb[:],
                                     qkv_sb[:, slot, t0:t1],
                                     start=True, stop=True)
                    rpb = scrp.tile([128, 512], BF16, name="rpb", tag="rpb")
                    nc.scalar.copy(rpb[:], rp[:])
                    c1 = scrp.tile([128, 512], BF16, name="ropec", tag="ropec")
                    nc.vector.tensor_mul(c1[:], qkv_sb[:, slot, t0:t1],
                                         cos_sb[:, t0:t1])
                    nc.vector.tensor_mul(rpb[:], rpb[:], sin_sb[:, t0:t1])
                    nc.vector.tensor_add(qkv_sb[:, slot, t0:t1],
                                         c1[:], rpb[:])
                # v -> token-major via PE transpose (this chunk's tokens)
                for hh in range(2):
                    for tt in range(t0 // 128, t1 // 128):
                        tp = mps.tile([128, 128], BF16, name="tps", tag="tps")
                        nc.tensor.transpose(
                            tp[:],
                            qkv_sb[:, 4 + hh, tt * 128:tt * 128 + 128],
                            ident[:])
                        nc.vector.tensor_copy(
                            v_sb[:, tt, hh * 128:hh * 128 + 128], tp[:])
        hps.close()  # h_sb no longer needed; frees 8.4MB for prefetch

        # ===== phase B: self-attention (causal skip, mult. diag masks) =====
        mrow = {}
        _mr = 0
        for ci in range(4):
            for kt in range(16):
                if blk[ci * 16 + kt] == 2:
                    mrow[(ci, kt)] = _mr
                    _mr += 1
        with ExitStack() as pB:
            ap_ = pB.enter_context(tc.tile_pool(name="ap", bufs=3))
            aps = pB.enter_context(tc.tile_pool(name="aps", bufs=2,
                                                space="PSUM"))
            accp = pB.enter_context(tc.tile_pool(name="accp", bufs=1,
                                                 space="PSUM"))
            for ci, (t0, t1) in enumerate(_chunks(0, S, 512)):
                live = [(kt, blk[ci * 16 + kt]) for kt in range(16)
                        if blk[ci * 16 + kt] != 0]
                pss_ = [accp.tile([128, 512], F32, name=f"pbs{h}",
                                  tag=f"pbs{h}") for h in range(2)]
                psc_ = [accp.tile([128, 512], F32, name=f"pbc{h}",
                                  tag=f"pbc{h}") for h in range(2)]
                its = [(kt, st, hh) for kt, st in live for hh in range(2)]
                sc_t, mt_t = {}, {}

                def emit_sc(i, ci=ci, t0=t0, t1=t1, its=its, sc_t=sc_t,
                            mt_t=mt_t):
                    kt, st, hh = its[i]
                    if st == 2 and hh == 0:
                        mt_ = ap_.tile([128, 512], BF16, name="mt", tag="mt")
                        r0 = mrow[(ci, kt)] * 128
                        nc.sync.dma_start(mt_[:], maskd.ap()[r0:r0 + 128, :])
                        mt_t[kt] = mt_
                    sc = aps.tile([128, 512], F32, name=f"sc{ci}_{i}",
                                  tag="sc")
                    nc.tensor.matmul(
                        sc[:], qkv_sb[:, 2 + hh, kt * 128:kt * 128 + 128],
                        qkv_sb[:, hh, t0:t1], start=True, stop=True)
                    sc_t[i] = sc

                emit_sc(0)
                n_it = len(its)
                for i, (kt, st, hh) in enumerate(its):
                    if i + 1 < n_it:
                        emit_sc(i + 1)
                    sc = sc_t.pop(i)
                    pr = ap_.tile([128, 512], BF16, name="pr", tag="pr")
                    nc.scalar.activation(pr[:], sc[:], EXP, scale=SC)
                    if st == 2:
                        nc.vector.tensor_mul(pr[:], pr[:], mt_t[kt])
                    nc.tensor.matmul(pss_[hh][:], ones_bf[:], pr[:],
                                     start=(i < 2), stop=(i >= n_it - 2))
                    nc.tensor.matmul(
                        psc_[hh][:],
                        v_sb[:, kt, hh * 128:hh * 128 + 128],
                        pr[:], start=(i < 2), stop=(i >= n_it - 2))
                for hh in range(2):
                    rc = ap_.tile([128, 512], F32, name="rc", tag="rc")
                    nc.vector.reciprocal_approx_fast(rc[:], pss_[hh][:])
                    nc.vector.tensor_mul(ctx_sb[:, hh, t0:t1],
                                         psc_[hh][:], rc[:])
                for hh in range(2):
                    for j in (2 * ci, 2 * ci + 1):
                        nc.sync.dma_start(
                            a2ain[256 * j + 128 * hh:
                                  256 * j + 128 * hh + 128, :],
                            ctx_sb[:, hh, 256 * j:256 * j + 256])
        pAB.close()
        nc.gpsimd.collective_compute(
            "AllToAll", mybir.AluOpType.bypass, replica_groups=RG,
            ins=[a2ain.opt()], outs=[a2aout.opt()])

        # ===== phase C: dense (token-parallel) + h1 + rmsnorm + cq =====
        pCD = top.enter_context(ExitStack())
        cd = pCD.enter_context(tc.tile_pool(name="cd", bufs=1))
        h1_sb = cd.tile([128, 16, 256], F32)
        cq_sb = cd.tile([128, 8, 256], BF16)
        cctx_sb = cd.tile([128, 8, 256], BF16)
        dps = pCD.enter_context(tc.tile_pool(name="dps", bufs=2,
                                             space="PSUM"))
        kpre = pCD.enter_context(tc.tile_pool(name="kpre", bufs=1))
        k_sb = kpre.tile([128, 8, E], BF16)
        with ExitStack() as pC:
            cp = pC.enter_context(tc.tile_pool(name="cp", bufs=1))
            wdp = pC.enter_context(tc.tile_pool(name="wdp", bufs=2))
            cx_sb = cp.tile([128, 16, 256], BF16)
            nc.sync.dma_start(cx_sb[:], r128(a2aout[:]))
            re_sb = cp.tile([128, 16, 256], BF16)
            nc.sync.dma_start(re_sb[:], r128(resid.ap()))
            nc.sync.dma_start(k_sb[:], r128(kag[:]))  # prefetch for phase D
            wcq_sb = cp.tile([128, 16, CC], BF16)
            if not uniform:
                mv_sb = cp.tile([128, 256], BF16)
                nc.sync.dma_start(mv_sb[:], mv_in.ap()[:])
                ml_sb = cp.tile([128, 256], BF16)
                nc.sync.dma_start(ml_sb[:], ml_in.ap()[:])
                cxv = cp.tile([128, 16, 256], BF16)
                cxl = cp.tile([128, 16, 256], BF16)
                for kc in range(16):
                    nc.vector.tensor_mul(cxv[:, kc, :], cx_sb[:, kc, :],
                                         mv_sb[:])
                    nc.vector.tensor_mul(cxl[:, kc, :], cx_sb[:, kc, :],
                                         ml_sb[:])
            for mg in range(4):  # stream dense weight in 4 x 2.1MB tiles
                wsrc0 = wde if uniform else wde0
                wde_t = wdp.tile([128, 16, 512], BF16, name=f"wde{mg}",
                                 tag="wde")
                nc.sync.dma_start(
                    wde_t[:], r128(wsrc0.ap()[:, mg * 512:mg * 512 + 512]))
                if not uniform:
                    wde1_t = wdp.tile([128, 16, 512], BF16, name=f"wdeb{mg}",
                                      tag="wdeb")
                    nc.sync.dma_start(
                        wde1_t[:],
                        r128(wde1.ap()[:, mg * 512:mg * 512 + 512]))
                for mi in range(4):
                    mt = mg * 4 + mi
                    ps = dps.tile([128, 256], F32, name="dp", tag="dp")
                    for kc in range(16):
                        nc.tensor.matmul(
                            ps[:], wde_t[:, kc, mi * 128:mi * 128 + 128],
                            (cx_sb if uniform else cxv)[:, kc, :],
                            start=(kc == 0), stop=(kc == 15))
                    if uniform:
                        nc.vector.tensor_add(h1_sb[:, mt, :], ps[:],
                                             re_sb[:, mt, :])
                    else:
                        ps1 = dps.tile([128, 256], F32, name="dp1", tag="dp1")
                        for kc in range(16):
                            nc.tensor.matmul(
                                ps1[:],
                                wde1_t[:, kc, mi * 128:mi * 128 + 128],
                                cxl[:, kc, :],
                                start=(kc == 0), stop=(kc == 15))
                        t_ = scrp.tile([128, 256], F32, name="dt", tag="dt")
                        nc.vector.tensor_add(t_[:], ps[:], ps1[:])
                        nc.vector.tensor_add(h1_sb[:, mt, :], t_[:],
                                             re_sb[:, mt, :])
            nc.sync.dma_start(wcq_sb[:], r128(wcq.ap()))
            # rmsnorm h1 -> h1n (bf16)
            h1n_sb = cp.tile([128, 16, 256], BF16)
            pss = dps.tile([128, 256], F32, name="nps2", tag="dp")
            for kc in range(16):
                sq = scrp.tile([128, 256], BF16, name="sqc", tag="sqc")
                nc.scalar.activation(sq[:], h1_sb[:, kc, :], SQ)
                nc.tensor.matmul(pss[:], ones_bf[:], sq[:],
                                 start=(kc == 0), stop=(kc == 15))
            rms = scrp.tile([128, 256], F32, name="rmsc", tag="rmsc")
            nc.scalar.activation(rms[:], pss[:], SQRT,
                                 scale=1.0 / H, bias=eps_sb[:])
            rinv = scrp.tile([128, 256], F32, name="rinvc", tag="rmsc")
            nc.vector.reciprocal_approx_fast(rinv[:], rms[:])
            for kc in range(16):
                nc.vector.tensor_mul(h1n_sb[:, kc, :], h1_sb[:, kc, :],
                                     rinv[:])
            for mt in range(8):
                ps = dps.tile([128, 256], F32, name="cqp", tag="dp")
                for kc in range(16):
                    nc.tensor.matmul(ps[:],
                                     wcq_sb[:, kc, mt * 128:mt * 128 + 128],
                                     h1n_sb[:, kc, :],
                                     start=(kc == 0), stop=(kc == 15))
                nc.vector.tensor_copy(cq_sb[:, mt, :], ps[:])

        # ===== phase D: cross attention (token-parallel) + cdense =====
        with ExitStack() as pD:
            kp = pD.enter_context(tc.tile_pool(name="kp", bufs=1))
            v_sb2 = kp.tile([128, 16, CC], BF16)
            for r in range(NC_):
                nc.sync.dma_start(
                    v_sb2[:, :, r * 128:r * 128 + 128],
                    vag[r * E:(r + 1) * E, :].rearrange(
                        "(k p) d -> p k d", p=128))
            wcd_sb = kp.tile([128, 8, H], BF16)
            nc.sync.dma_start(wcd_sb[:], r128(wcd.ap()))
            cap = pD.enter_context(tc.tile_pool(name="cap", bufs=3))
            caps = pD.enter_context(tc.tile_pool(name="caps", bufs=2,
                                                 space="PSUM"))
            cacc = pD.enter_context(tc.tile_pool(name="cacc", bufs=1,
                                                 space="PSUM"))
            dits = [(m, kp) for m in range(8) for kp in range(8)]
            dsc_t = {}

            def emit_dsc(idx, dits=dits, dsc_t=dsc_t):
                m, kp = dits[idx]
                # separate PSUM banks per head: concurrent row-group
                # matmuls must not share a bank
                sca = caps.tile([128, 2, 256], F32, name=f"csa{idx}",
                                tag="csa")
                scb = caps.tile([128, 2, 256], F32, name=f"csb{idx}",
                                tag="csb")
                for j, kt in ((0, 2 * kp), (1, 2 * kp + 1)):
                    nc.tensor.matmul(
                        sca[:, j, :],
                        k_sb[0:64, m, kt * 128:kt * 128 + 128],
                        cq_sb[0:64, m, :], start=True, stop=True)
                    nc.tensor.matmul(
                        scb[:, j, :],
                        k_sb[64:128, m, kt * 128:kt * 128 + 128],
                        cq_sb[64:128, m, :], start=True, stop=True)
                dsc_t[idx] = (sca, scb)

            emit_dsc(0)
            psden = pc2 = None
            for idx, (m, kp) in enumerate(dits):
                if kp == 0:
                    # psden slot 0 = head-a denominator, slot 1 = head-b
                    psden = cacc.tile([128, 2, 256], F32, name=f"cps{m}",
                                      tag="cps")
                    pc2 = cacc.tile([128, 256], F32, name=f"cpc{m}",
                                    tag="cpc")
                if idx + 1 < len(dits):
                    emit_dsc(idx + 1)
                sca, scb = dsc_t.pop(idx)
                pra = cap.tile([128, 2, 256], BF16, name=f"cpra{idx}",
                               tag="cpra")
                nc.scalar.activation(pra[:], sca[:], EXP, scale=CSC)
                prb = cap.tile([128, 2, 256], BF16, name=f"cprb{idx}",
                               tag="cprb")
                nc.scalar.activation(prb[:], scb[:], EXP, scale=CSC)
                for j, kt in ((0, 2 * kp), (1, 2 * kp + 1)):
                    nc.tensor.matmul(psden[:, 0, :], ones_bf[:],
                                     pra[:, j, :],
                                     start=(kp == 0 and j == 0),
                                     stop=(kp == 7 and j == 1))
                    nc.tensor.matmul(psden[:, 1, :], ones_bf[:],
                                     prb[:, j, :],
                                     start=(kp == 0 and j == 0),
                                     stop=(kp == 7 and j == 1))
                    nc.tensor.matmul(
                        pc2[0:64, :], v_sb2[:, kt, 128 * m:128 * m + 64],
                        pra[:, j, :], start=(kp == 0 and j == 0),
                        stop=(kp == 7 and j == 1))
                    nc.tensor.matmul(
                        pc2[64:128, :],
                        v_sb2[:, kt, 128 * m + 64:128 * m + 128],
                        prb[:, j, :], start=(kp == 0 and j == 0),
                        stop=(kp == 7 and j == 1))
                if kp == 7:
                    rc = cap.tile([64, 2, 256], F32, name=f"crc{m}",
                                  tag="crc")
                    nc.vector.reciprocal_approx_fast(rc[:], psden[0:64, :, :])
                    nc.vector.tensor_mul(cctx_sb[0:64, m, :], pc2[0:64, :],
                                         rc[:, 0, :])
                    nc.vector.tensor_mul(cctx_sb[64:128, m, :],
                                         pc2[64:128, :], rc[:, 1, :])
            # cdense + residual -> h2, rmsnorm -> h2n -> AG
            d4 = pD.enter_context(tc.tile_pool(name="d4", bufs=1))
            h2_sb = d4.tile([128, 16, 256], F32)
            h2n_sb = d4.tile([128, 16, 256], BF16)
            for mt in range(16):
                ps = dps.tile([128, 256], F32, name="cdp", tag="dp")
                for kc in range(8):
                    nc.tensor.matmul(ps[:],
                                     wcd_sb[:, kc, mt * 128:mt * 128 + 128],
                                     cctx_sb[:, kc, :],
                                     start=(kc == 0), stop=(kc == 7))
                nc.vector.tensor_add(h2_sb[:, mt, :], ps[:],
                                     h1_sb[:, mt, :])
            pss2 = dps.tile([128, 256], F32, name="psd2", tag="dp")
            for kc in range(16):
                sq = scrp.tile([128, 256], BF16, name="sqd2", tag="sqc")
                nc.scalar.activation(sq[:], h2_sb[:, kc, :], SQ)
                nc.tensor.matmul(pss2[:], ones_bf[:], sq[:],
                                 start=(kc == 0), stop=(kc == 15))
            rms2 = scrp.tile([128, 256], F32, name="rmsd2", tag="rmsc")
            nc.scalar.activation(rms2[:], pss2[:], SQRT,
                                 scale=1.0 / H, bias=eps_sb[:])
            rinv2 = scrp.tile([128, 256], F32, name="rinvd", tag="rmsc")
            nc.vector.reciprocal_approx_fast(rinv2[:], rms2[:])
            for kc in range(16):
                nc.vector.tensor_mul(h2n_sb[:, kc, :],
                                     h2_sb[:, kc, :], rinv2[:])
            nc.sync.dma_start(r128(h2nb[:]), h2n_sb[:])
            nc.sync.dma_start(r128(h2out.ap()), h2_sb[:])
        pCD.close()
        nc.gpsimd.collective_compute(
            "AllGather", mybir.AluOpType.bypass, replica_groups=RG,
            ins=[h2nb.opt()], outs=[h2na.opt()])

        # ===== phase F: MLP (routed by expert ranges, bf16) =====
        with ExitStack() as pF:
            fp = pF.enter_context(tc.tile_pool(name="fp", bufs=1))
            hn_sb = fp.tile([128, 16, S], BF16)
            for r in range(NC_):
                nc.sync.dma_start(hn_sb[:, :, r * 256:r * 256 + 256],
                                  r128(h2na[r * H:(r + 1) * H, :]))
            fw = pF.enter_context(tc.tile_pool(name="fw", bufs=1))
            fps = pF.enter_context(tc.tile_pool(name="fps", bufs=2,
                                                space="PSUM"))
            fpd = pF.enter_context(tc.tile_pool(name="fpd", bufs=2,
                                                space="PSUM"))
            fac = pF.enter_context(tc.tile_pool(name="fac", bufs=2))
            fout = pF.enter_context(tc.tile_pool(name="fout", bufs=4))
            for ex, (lo, hi) in ((0, (0, b1)), (1, (b1, S))):
                gsrc = (wgu0, wgu1)[ex]
                dsrc = (wdn0, wdn1)[ex]
                dn_t = fw.tile([128, 6, H], BF16, name=f"dn{ex}", tag="dn")
                nc.sync.dma_start(dn_t[:], r128(dsrc.ap()))
                gwts = []
                for pi in range(6):
                    gw = 128 if pi < 5 else 48
                    gwt = fw.tile([128, 16, 256], BF16,
                                  name=f"guw{ex}{pi}", tag=f"guw{pi}")
                    nc.sync.dma_start(
                        gwt[:, :, :2 * gw],
                        r128(gsrc.ap()[:, pi * 256:pi * 256 + 2 * gw]))
                    gwts.append(gwt)
                for a0 in range(0, S, 512):
                    c0, c1 = max(a0, lo), min(a0 + 512, hi)
                    if c0 >= c1:
                        continue
                    t0_, W = a0, 512
                    eo, ew = c0 - a0, c1 - c0
                    act = fac.tile([128, 6, 512], BF16, name="act", tag="act")
                    for pi in range(6):
                        gw = 128 if pi < 5 else 48
                        gwt = gwts[pi]
                        pg = fps.tile([128, 512], F32, name="pg", tag="pg")
                        pu = fps.tile([128, 512], F32, name="pu", tag="pu")
                        for kc in range(16):
                            nc.tensor.matmul(pg[:gw, :W], gwt[:, kc, :gw],
                                             hn_sb[:, kc, t0_:t0_ + 512],
                                             start=(kc == 0), stop=(kc == 15))
                            nc.tensor.matmul(pu[:gw, :W], gwt[:, kc, gw:2 * gw],
                                             hn_sb[:, kc, t0_:t0_ + 512],
                                             start=(kc == 0), stop=(kc == 15))
                        gs = scrp.tile([128, 512], F32, name="gs", tag="gs")
                        nc.scalar.activation(gs[:gw, :W], pg[:gw, :W], SILU)
                        nc.vector.tensor_mul(act[:gw, pi, :W],
                                             gs[:gw, :W], pu[:gw, :W])
                    for mt in range(16):
                        pd = fpd.tile([128, 512], F32, name="pd", tag="pd")
                        for pi in range(6):
                            kw = 128 if pi < 5 else 48
                            nc.tensor.matmul(
                                pd[:, :W],
                                dn_t[:kw, pi, mt * 128:mt * 128 + 128],
                                act[:kw, pi, :W],
                                start=(pi == 0), stop=(pi == 5))
                        ot = fout.tile([128, 512], F32, name="fot", tag="fot")
                        if mt % 2 == 0:
                            nc.vector.tensor_copy(ot[:, eo:eo + ew],
                                                  pd[:, eo:eo + ew])
                        else:
                            nc.scalar.copy(ot[:, eo:eo + ew],
                                           pd[:, eo:eo + ew])
                        nc.sync.dma_start(
                            y.ap()[mt * 128:mt * 128 + 128, c0:c1],
                            ot[:, eo:eo + ew])
    nc.compile()
    return nc


_CACHE = {}


def kernel(**inputs):
    import ml_dtypes
    vm = np.asarray(inputs["vision_token_ids"]).astype(bool)
    lm = np.asarray(inputs["language_token_ids"]).astype(bool)
    g0 = np.where(vm & ~lm)[0]; g1 = np.where(vm & lm)[0]
    g2 = np.where(~vm & lm)[0]; g3 = np.where(~vm & ~lm)[0]
    perm = np.concatenate([g0, g1, g2, g3])
    b0 = len(g0); b1 = b0 + len(g1); b2 = b1 + len(g2)

    f32 = lambda x: np.ascontiguousarray(np.asarray(x, np.float32))
    bf = lambda x: np.ascontiguousarray(np.asarray(x).astype(ml_dtypes.bfloat16))
    pos = np.asarray(inputs["positions"]).astype(np.float32)
    half = HD // 2
    inv_freq = 1.0 / (ROPE_BASE ** (np.arange(half, dtype=np.float32) / half))
    fr = pos[:, None] * inv_freq[None, :]
    cos2 = np.concatenate([np.cos(fr)] * 2, 1).T[:, perm]
    sin2 = np.concatenate([np.sin(fr)] * 2, 1).T[:, perm]
    rot = np.zeros((HD, HD), np.float32)
    rot[np.arange(half), np.arange(half) + half] = -1.0
    rot[np.arange(half) + half, np.arange(half)] = 1.0
    op = np.asarray(inputs["positions"])[perm]

    # causal block states + diagonal multiplicative masks
    blk = []
    mrows = []
    for ci in range(4):
        qv = op[512 * ci:512 * ci + 512]
        for kt in range(16):
            kv = op[128 * kt:128 * kt + 128]
            if kv.max() <= qv.min():
                blk.append(1)
            elif kv.min() > qv.max():
                blk.append(0)
            else:
                blk.append(2)
                mrows.append((qv[None, :] >= kv[:, None]).astype(np.float32))
    blk = tuple(blk)
    maskd = (np.concatenate(mrows, 0) if mrows
             else np.zeros((128, 512), np.float32))

    # per-chunk expert combos (0=vis, 1=both, 2=lang, 3=neither)
    combo = np.full(S, 3, np.int8)
    combo[:b0] = 0; combo[b0:b1] = 1; combo[b1:b2] = 2
    chunk_combo = []
    uniform = True
    for j in range(NC_):
        cj = combo[256 * j:256 * j + 256]
        if (cj == cj[0]).all():
            chunk_combo.append(int(cj[0]))
        else:
            chunk_combo.append(-1)
            uniform = False

    wln_in = f32(inputs["w_ln_in"])[:, None]
    wln_pa = f32(inputs["w_ln_post_attn"])[:, None]
    wln_pc = f32(inputs["w_ln_post_cross"])[:, None]
    wqkv = [f32(inputs["w_vis_qkv"]) * wln_in, f32(inputs["w_lang_qkv"]) * wln_in]
    wd = [f32(inputs["w_vis_dense"]), f32(inputs["w_lang_dense"])]
    wgu = [f32(inputs["w_vis_gate_up"]) * wln_pc,
           f32(inputs["w_lang_gate_up"]) * wln_pc]
    wdn = [f32(inputs["w_vis_down"]), f32(inputs["w_lang_down"])]
    wkvf = f32(inputs["w_cross_kv"])
    hTp = f32(inputs["hidden_states"]).T[:, perm].copy()

    def interleave(w):  # w [H, 2*IS] = [gate | up]
        cols = []
        for i in range(5):
            cols.append(w[:, 128 * i:128 * i + 128])
            cols.append(w[:, IS + 128 * i:IS + 128 * i + 128])
        cols.append(w[:, 640:IS]); cols.append(w[:, IS + 640:2 * IS])
        return np.ascontiguousarray(np.concatenate(cols, 1))

    key = (b0, b1, b2, blk, uniform)
    if key not in _CACHE:
        _CACHE.clear()
        _CACHE[key] = build_kernel(b0, b1, b2, blk, uniform)
    nc = _CACHE[key]

    # dense weight combos (bf16, built once per distinct combo)
    wde_by_combo = {}
    for cb in set(chunk_combo):
        if cb == 0:
            wde_by_combo[cb] = bf(wd[0])
        elif cb == 1:
            wde_by_combo[cb] = bf(wd[0] + wd[1])
        elif cb == 2:
            wde_by_combo[cb] = bf(wd[1])
        elif cb == 3:
            wde_by_combo[cb] = bf(np.zeros_like(wd[0]))

    in_maps = []
    for c in range(NC_):
        qs = slice(256 * c, 256 * c + 256)
        m = dict(
            hT=bf(hTp),
            resid=bf(hTp[:, qs]),
            wqkv0=bf(np.concatenate([wqkv[0][:, qs], wqkv[0][:, H:][:, qs],
                                     wqkv[0][:, 2 * H:][:, qs]], 1)),
            wqkv1=bf(np.concatenate([wqkv[1][:, qs], wqkv[1][:, H:][:, qs],
                                     wqkv[1][:, 2 * H:][:, qs]], 1)),
            cos2=bf(cos2), sin2=bf(sin2), rotT=bf(rot.T),
            onesb=np.ones((128, 128), ml_dtypes.bfloat16),
            maskd=bf(maskd),
            encT=bf(f32(inputs["encoder_embeds"]).T),
            wkc=bf(wkvf[:, 128 * c:128 * c + 128]),
            wvc=bf(wkvf[:, CC + 128 * c:CC + 128 * c + 128]),
            wcq=bf(f32(inputs["w_cross_q"]) * wln_pa),
            wcd=bf(f32(inputs["w_cross_dense"])),
            wgu0=bf(interleave(np.concatenate(
                [wgu[0][:, IS * c:IS * c + IS],
                 wgu[0][:, I + IS * c:I + IS * c + IS]], 1))),
            wgu1=bf(interleave(np.concatenate(
                [wgu[1][:, IS * c:IS * c + IS],
                 wgu[1][:, I + IS * c:I + IS * c + IS]], 1))),
            wdn0=bf(np.concatenate([wdn[0][IS * c:IS * c + IS],
                                    np.zeros((ISP - IS, H), np.float32)], 0)),
            wdn1=bf(np.concatenate([wdn[1][IS * c:IS * c + IS],
                                    np.zeros((ISP - IS, H), np.float32)], 0)),
        )
        if uniform:
            m["wde"] = wde_by_combo[chunk_combo[c]]
        else:
            m["wde0"] = bf(wd[0])
            m["wde1"] = bf(wd[1])
            pv = vm[perm][qs].astype(np.float32)
            pl = lm[perm][qs].astype(np.float32)
            m["mv"] = bf(np.broadcast_to(pv[None, :], (128, 256)).copy())
            m["ml"] = bf(np.broadcast_to(pl[None, :], (128, 256)).copy())
        in_maps.append(m)

    # wqkv slot layout check: slots are [q(2x128) | k(2x128) | v(2x128)]
    # per-core head pair -> columns 128c..128c+256 of each of q,k,v.

    if os.environ.get("KSIM"):
        from concourse.bass_interp import MultiCoreSim
        sim = MultiCoreSim(nc, num_cores=NC_)
        for c, cs in sim.cores.items():
            for name, val in in_maps[c].items():
                cs.tensor(name)[:] = val
        sim.simulate(check_with_hw=False)
        results = [dict(y=np.array(sim.cores[c].tensor("y")),
                        h2out=np.array(sim.cores[c].tensor("h2out")))
                   for c in range(NC_)]
        kernel.last_exec_ns = 0
    else:
        trace = bool(int(os.environ.get("KTRACE", "0")))
        res = run_bass_kernel_spmd(nc, in_maps, core_ids=list(range(NC_)),
                                   trace=trace)
        kernel.last_exec_ns = res.exec_time_ns
        results = res.results
    tot = results[0]["y"].astype(np.float64)
    for c in range(1, NC_):
        tot += results[c]["y"]
    for c in range(NC_):
        tot[:, 256 * c:256 * c + 256] += results[c]["h2out"]
    out = np.empty((S, H), np.float32)
    out[perm, :] = tot.T.astype(np.float32)
    return out


# revision 29
# speedup vs baseline: 1.1925x; 1.0010x over previous
"""Trainium2 Bass kernel for nn_CogAgentDecoderLayer (8-core SPMD).

Feature-major activations [feat, tok] in permuted token order
(vis-only | both | lang-only | neither). TP: QKV/self-attn by heads
(2/core), MLP by intermediate slice (688/core). Cross-attn K/V computed
sharded (128 of 1024 dims per core) + AllGathered early, overlapped with
self-attention. Self-attn ctx redistributed with AllToAll (head-shards ->
token-shards), then dense/cross-attn/cdense run token-parallel (256
tok/core). Final MLP partial sums reduced on host. Self-attention skips
fully-masked causal blocks; diagonal blocks use multiplicative 0/1 masks.
bf16 matmuls throughout, fp32 psum/residual/norm stats.
"""
import os
import sys
import types
import numpy as np
from contextlib import ExitStack

# concourse's trace path does `from antenv.axon_hooks import ...`; provide a
# stub registry if the module is missing so tracing degrades instead of
# crashing (the boot hook registers itself here when available).
try:
    import antenv.axon_hooks  # noqa: F401
except Exception:
    try:
        import antenv
        _m = types.ModuleType("antenv.axon_hooks")
        _m._hook = None
        _m.set_axon_ntff_profile_hook = lambda h: setattr(_m, "_hook", h)
        _m.get_axon_ntff_profile_hook = lambda: _m._hook
        sys.modules["antenv.axon_hooks"] = _m
        antenv.axon_hooks = _m
    except Exception:
        pass

from concourse import bacc, tile, mybir
from concourse.bass_utils import run_bass_kernel_spmd

NC_ = 8
S, E, H, NH, HD = 2048, 2048, 2048, 16, 128
CH, CC, CHD = 1024, 1024, 64
I = 5504
IS = I // NC_          # 688
ISP = 768              # padded to 6*128
EPS = 1e-5
ROPE_BASE = 10000.0
F32 = mybir.dt.float32
F32R = mybir.dt.float32r
BF16 = mybir.dt.bfloat16


def _segs(lo, hi, b0, b1, b2):
    pts = sorted({lo, hi, *[b for b in (b0, b1, b2) if lo < b < hi]})
    out = []
    for s, e in zip(pts, pts[1:]):
        ex = []
        if s < b1:
            ex.append(0)
        if b0 <= s < b2:
            ex.append(1)
        out.append((s, e, ex))
    return out


def _chunks(lo, hi, w):
    out = []
    while lo < hi:
        out.append((lo, min(lo + w, hi)))
        lo += w
    return out


def build_kernel(b0, b1, b2, blk, uniform):
    """blk: 64-tuple, state per (ci, kt): 0=skip, 1=visible, 2=partial.
    uniform: every 256-token chunk has a single expert-combo (host packs
    the right dense weight per core)."""
    nc = bacc.Bacc("TRN2", target_bir_lowering=False, debug=False,
                   num_devices=NC_)
    din = lambda n, sh, dt: nc.dram_tensor(n, sh, dt, kind="ExternalInput")
    hT = din("hT", [H, S], BF16)
    resid = din("resid", [H, 256], BF16)
    wqkv0 = din("wqkv0", [H, 768], BF16)
    wqkv1 = din("wqkv1", [H, 768], BF16)
    cos2 = din("cos2", [128, S], BF16)
    sin2 = din("sin2", [128, S], BF16)
    rotT = din("rotT", [128, 128], BF16)
    onesb = din("onesb", [128, 128], BF16)
    nmask = max(1, sum(1 for st in blk if st == 2))
    maskd = din("maskd", [128 * nmask, 512], BF16)
    encT = din("encT", [CH, E], BF16)
    wkc = din("wkc", [CH, 128], BF16)
    wvc = din("wvc", [CH, 128], BF16)
    if uniform:
        wde = din("wde", [H, H], BF16)
    else:
        wde0 = din("wde0", [H, H], BF16)
        wde1 = din("wde1", [H, H], BF16)
        mv_in = din("mv", [128, 256], BF16)
        ml_in = din("ml", [128, 256], BF16)
    wcq = din("wcq", [H, CC], BF16)
    wcd = din("wcd", [CC, H], BF16)
    wgu0 = din("wgu0", [H, 2 * IS], BF16)
    wgu1 = din("wgu1", [H, 2 * IS], BF16)
    wdn0 = din("wdn0", [ISP, H], BF16)
    wdn1 = din("wdn1", [ISP, H], BF16)
    y = nc.dram_tensor("y", [H, S], F32, kind="ExternalOutput")
    h2out = nc.dram_tensor("h2out", [H, 256], F32, kind="ExternalOutput")

    SC = 1.0 / float(np.sqrt(HD))
    CSC = 1.0 / float(np.sqrt(CHD))
    EXP = mybir.ActivationFunctionType.Exp
    SQ = mybir.ActivationFunctionType.Square
    SQRT = mybir.ActivationFunctionType.Sqrt
    SILU = mybir.ActivationFunctionType.Silu
    r128 = lambda ap: ap.rearrange("(c p) n -> p c n", p=128)
    RG = [list(range(NC_))]

    with tile.TileContext(nc) as tc, ExitStack() as top:
        const = top.enter_context(tc.tile_pool(name="const", bufs=1))
        ones_bf = const.tile([128, 128], BF16)
        nc.sync.dma_start(ones_bf[:], onesb.ap()[:])
        rot_sb = const.tile([128, 128], BF16)
        nc.sync.dma_start(rot_sb[:], rotT.ap()[:])
        from concourse.masks import make_identity
        ident = const.tile([128, 128], BF16)
        make_identity(nc, ident[:])
        cos_sb = const.tile([128, S], BF16)
        sin_sb = const.tile([128, S], BF16)
        eps_sb = const.tile([128, 1], F32)
        nc.vector.memset(eps_sb[:], EPS)

        dram = top.enter_context(tc.tile_pool(name="dram", bufs=1, space="DRAM"))
        kbnc = dram.tile([128, E], BF16)
        vbnc = dram.tile([E, 128], BF16)
        kag = dram.tile([NC_ * 128, E], BF16, addr_space="Shared")
        vag = dram.tile([NC_ * E, 128], BF16, addr_space="Shared")
        a2ain = dram.tile([H, 256], BF16)
        a2aout = dram.tile([H, 256], BF16)
        h2nb = dram.tile([H, 256], BF16)
        h2na = dram.tile([NC_ * H, 256], BF16, addr_space="Shared")

        scrp = top.enter_context(tc.tile_pool(name="scr", bufs=2))

        # tiles for A/B; h DMA deferred until after phase-0 inputs so the
        # cross-KV compute (first PE work) isn't starved by the h transfer
        pAB = top.enter_context(ExitStack())
        qkp = pAB.enter_context(tc.tile_pool(name="qkp", bufs=1))
        qkv_sb = qkp.tile([128, 6, S], BF16)      # q0 q1 k0 k1 v0 v1
        v_sb = qkp.tile([128, 16, 256], BF16)     # token-major v
        ctx_sb = qkp.tile([128, 2, S], BF16)
        hps = top.enter_context(ExitStack())
        hp = hps.enter_context(tc.tile_pool(name="hp", bufs=1))
        h_sb = hp.tile([128, 16, S], BF16)

        # ===== phase 0: cross K/V shard compute + AllGathers =====
        with ExitStack() as p0:
            ep = p0.enter_context(tc.tile_pool(name="ep", bufs=1))
            enc_sb = ep.tile([128, 8, E], BF16)
            nc.sync.dma_start(enc_sb[:], r128(encT.ap()))
            wk_sb = ep.tile([128, 8, 128], BF16)
            nc.sync.dma_start(wk_sb[:], r128(wkc.ap()))
            wv_sb = ep.tile([128, 8, 128], BF16)
            nc.sync.dma_start(wv_sb[:], r128(wvc.ap()))
            kb_sb = ep.tile([128, E], BF16)
            vb_sb = ep.tile([128, 16, 128], BF16)
            kvp = p0.enter_context(tc.tile_pool(name="kvp", bufs=2,
                                                space="PSUM"))
            for n0, n1 in _chunks(0, E, 512):
                ps = kvp.tile([128, 512], F32, name="kps", tag="kps")
                for kc in range(8):
                    nc.tensor.matmul(ps[:], wk_sb[:, kc, :],
                                     enc_sb[:, kc, n0:n1],
                                     start=(kc == 0), stop=(kc == 7))
                nc.vector.tensor_copy(kb_sb[:, n0:n1], ps[:])
            for kt in range(16):
                ps = kvp.tile([128, 128], F32, name="vps", tag="vps")
                for kc in range(8):
                    nc.tensor.matmul(ps[:],
                                     enc_sb[:, kc, kt * 128:kt * 128 + 128],
                                     wv_sb[:, kc, :],
                                     start=(kc == 0), stop=(kc == 7))
                nc.vector.tensor_copy(vb_sb[:, kt, :], ps[:])
            nc.sync.dma_start(kbnc[:], kb_sb[:])
            nc.sync.dma_start(vbnc[:].rearrange("(k p) d -> p k d", p=128),
                              vb_sb[:])
        nc.gpsimd.collective_compute(
            "AllGather", mybir.AluOpType.bypass, replica_groups=RG,
            ins=[kbnc.opt()], outs=[kag.opt()])
        nc.gpsimd.collective_compute(
            "AllGather", mybir.AluOpType.bypass, replica_groups=RG,
            ins=[vbnc.opt()], outs=[vag.opt()])

        # ===== phase A: per-chunk rmsnorm + QKV + rope + vT =====
        with ExitStack() as pA2:
            wp = pA2.enter_context(tc.tile_pool(name="wp", bufs=1))
            wq_sb = [wp.tile([128, 16, 768], BF16, name=f"wqa{x}",
                             tag=f"wqa{x}") for x in range(2)]
            # DMA order tuned for earliest first matmul: h chunk 0, then
            # QKV weights, then the rest of h, then rope tables
            nc.sync.dma_start(
                h_sb[:, :, 0:512],
                hT.ap()[:, 0:512].rearrange("(c p) n -> p c n", p=128))
            nc.sync.dma_start(wq_sb[0][:], r128(wqkv0.ap()))
            nc.sync.dma_start(wq_sb[1][:], r128(wqkv1.ap()))
            for t0, t1 in _chunks(512, S, 512):
                nc.sync.dma_start(
                    h_sb[:, :, t0:t1],
                    hT.ap()[:, t0:t1].rearrange("(c p) n -> p c n", p=128))
            nc.sync.dma_start(cos_sb[:], cos2.ap()[:])
            nc.sync.dma_start(sin_sb[:], sin2.ap()[:])
            nrm = pA2.enter_context(tc.tile_pool(name="nrm", bufs=2))
            mps = pA2.enter_context(tc.tile_pool(name="mps", bufs=2,
                                                 space="PSUM"))
            for ci, (t0, t1) in enumerate(_chunks(0, S, 512)):
                pss = mps.tile([128, 512], F32, name="pss", tag="qps1")
                for kc in range(16):
                    sq = nrm.tile([128, 512], BF16, name="sq", tag="sq")
                    nc.scalar.activation(sq[:], h_sb[:, kc, t0:t1], SQ)
                    nc.tensor.matmul(pss[:], ones_bf[:], sq[:],
                                     start=(kc == 0), stop=(kc == 15))
                rms = nrm.tile([128, 512], F32, name="rms", tag="rms")
                nc.scalar.activation(rms[:], pss[:], SQRT,
                                     scale=1.0 / H, bias=eps_sb[:])
                rinv = nrm.tile([128, 512], F32, name="rinv", tag="rinv")
                nc.vector.reciprocal_approx_fast(rinv[:], rms[:])
                rinvb = nrm.tile([128, 512], BF16, name="rinvb", tag="rinvb")
                nc.vector.tensor_copy(rinvb[:], rinv[:])
                for kc in range(16):
                    nc.vector.tensor_mul(h_sb[:, kc, t0:t1],
                                         h_sb[:, kc, t0:t1], rinvb[:])
                sg = [x for x in _segs(t0, t1, b0, b1, b2) if x[2]]
                for slot in range(6):
                    if sg:
                        need = sorted({x for _, _, ex in sg for x in ex})
                        pss_ = {}
                        for x in need:
                            ps = mps.tile([128, 512], F32, name=f"qps{x}",
                                          tag=f"qps{x}")
                            for kc in range(16):
                                nc.tensor.matmul(
                                    ps[:],
                                    wq_sb[x][:, kc, slot * 128:slot * 128 + 128],
                                    h_sb[:, kc, t0:t1],
                                    start=(kc == 0), stop=(kc == 15))
                            pss_[x] = ps
                        for s, e, ex in sg:
                            if len(ex) == 1:
                                nc.vector.tensor_copy(
                                    qkv_sb[:, slot, s:e],
                                    pss_[ex[0]][:, s - t0:e - t0])
                            else:
                                nc.vector.tensor_add(qkv_sb[:, slot, s:e],
                                                     pss_[0][:, s - t0:e - t0],
                                                     pss_[1][:, s - t0:e - t0])
                    if t1 > b2 > t0:
                        nc.vector.memset(qkv_sb[:, slot, b2:t1], 0.0)
                    elif t0 >= b2:
                        nc.vector.memset(qkv_sb[:, slot, t0:t1], 0.0)
                # rope on q,k of this chunk
                for slot in range(4):
                    rp = mps.tile([128, 512], F32, name="rps", tag="qps0")
                    nc.tensor.matmul(rp[:], rot_sb[:],
                                     qkv_sb[:, slot, t0:t1],
                                     start=True, stop=True)
                    rpb = scrp.tile([128, 512], BF16, name="rpb", tag="rpb")
                    nc.scalar.copy(rpb[:], rp[:])
                    c1 = scrp.tile([128, 512], BF16, name="ropec", tag="ropec")
                    nc.vector.tensor_mul(c1[:], qkv_sb[:, slot, t0:t1],
                                         cos_sb[:, t0:t1])
                    nc.vector.tensor_mul(rpb[:], rpb[:], sin_sb[:, t0:t1])
                    nc.vector.tensor_add(qkv_sb[:, slot, t0:t1],
                                         c1[:], rpb[:])
                # v -> token-major via PE transpose (this chunk's tokens)
                for hh in range(2):
                    for tt in range(t0 // 128, t1 // 128):
                        tp = mps.tile([128, 128], BF16, name="tps", tag="tps")
                        nc.tensor.transpose(
                            tp[:],
                            qkv_sb[:, 4 + hh, tt * 128:tt * 128 + 128],
                            ident[:])
                        nc.vector.tensor_copy(
                            v_sb[:, tt, hh * 128:hh * 128 + 128], tp[:])
        hps.close()  # h_sb no longer needed; frees 8.4MB for prefetch

        # ===== phase B: self-attention (causal skip, mult. diag masks) =====
        mrow = {}
        _mr = 0
        for ci in range(4):
            for kt in range(16):
                if blk[ci * 16 + kt] == 2:
                    mrow[(ci, kt)] = _mr
                    _mr += 1
        with ExitStack() as pB:
            ap_ = pB.enter_context(tc.tile_pool(name="ap", bufs=3))
            aps = pB.enter_context(tc.tile_pool(name="aps", bufs=2,
                                                space="PSUM"))
            accp = pB.enter_context(tc.tile_pool(name="accp", bufs=1,
                                                 space="PSUM"))
            for ci, (t0, t1) in enumerate(_chunks(0, S, 512)):
                live = [(kt, blk[ci * 16 + kt]) for kt in range(16)
                        if blk[ci * 16 + kt] != 0]
                pss_ = [accp.tile([128, 512], F32, name=f"pbs{h}",
                                  tag=f"pbs{h}") for h in range(2)]
                psc_ = [accp.tile([128, 512], F32, name=f"pbc{h}",
                                  tag=f"pbc{h}") for h in range(2)]
                its = [(kt, st, hh) for kt, st in live for hh in range(2)]
                sc_t, mt_t = {}, {}

                def emit_sc(i, ci=ci, t0=t0, t1=t1, its=its, sc_t=sc_t,
                            mt_t=mt_t):
                    kt, st, hh = its[i]
                    if st == 2 and hh == 0:
                        mt_ = ap_.tile([128, 512], BF16, name="mt", tag="mt")
                        r0 = mrow[(ci, kt)] * 128
                        nc.sync.dma_start(mt_[:], maskd.ap()[r0:r0 + 128, :])
                        mt_t[kt] = mt_
                    sc = aps.tile([128, 512], F32, name=f"sc{ci}_{i}",
                                  tag="sc")
                    nc.tensor.matmul(
                        sc[:], qkv_sb[:, 2 + hh, kt * 128:kt * 128 + 128],
                        qkv_sb[:, hh, t0:t1], start=True, stop=True)
                    sc_t[i] = sc

                emit_sc(0)
                n_it = len(its)
                for i, (kt, st, hh) in enumerate(its):
                    if i + 1 < n_it:
                        emit_sc(i + 1)
                    sc = sc_t.pop(i)
                    pr = ap_.tile([128, 512], BF16, name="pr", tag="pr")
                    nc.scalar.activation(pr[:], sc[:], EXP, scale=SC)
                    if st == 2:
                        nc.vector.tensor_mul(pr[:], pr[:], mt_t[kt])
                    nc.tensor.matmul(pss_[hh][:], ones_bf[:], pr[:],
                                     start=(i < 2), stop=(i >= n_it - 2))
                    nc.tensor.matmul(
                        psc_[hh][:],
                        v_sb[:, kt, hh * 128:hh * 128 + 128],
                        pr[:], start=(i < 2), stop=(i >= n_it - 2))
                for hh in range(2):
                    rc = ap_.tile([128, 512], F32, name="rc", tag="rc")
                    nc.vector.reciprocal_approx_fast(rc[:], pss_[hh][:])
                    nc.vector.tensor_mul(ctx_sb[:, hh, t0:t1],
                                         psc_[hh][:], rc[:])
                for hh in range(2):
                    for j in (2 * ci, 2 * ci + 1):
                        nc.sync.dma_start(
                            a2ain[256 * j + 128 * hh:
                                  256 * j + 128 * hh + 128, :],
                            ctx_sb[:, hh, 256 * j:256 * j + 256])
        pAB.close()
        nc.gpsimd.collective_compute(
            "AllToAll", mybir.AluOpType.bypass, replica_groups=RG,
            ins=[a2ain.opt()], outs=[a2aout.opt()])

        # ===== phase C: dense (token-parallel) + h1 + rmsnorm + cq =====
        pCD = top.enter_context(ExitStack())
        cd = pCD.enter_context(tc.tile_pool(name="cd", bufs=1))
        h1_sb = cd.tile([128, 16, 256], F32)
        cq_sb = cd.tile([128, 8, 256], BF16)
        cctx_sb = cd.tile([128, 8, 256], BF16)
        dps = pCD.enter_context(tc.tile_pool(name="dps", bufs=2,
                                             space="PSUM"))
        kpre = pCD.enter_context(tc.tile_pool(name="kpre", bufs=1))
        k_sb = kpre.tile([128, 8, E], BF16)
        with ExitStack() as pC:
            cp = pC.enter_context(tc.tile_pool(name="cp", bufs=1))
            wdp = pC.enter_context(tc.tile_pool(name="wdp", bufs=2))
            cx_sb = cp.tile([128, 16, 256], BF16)
            nc.sync.dma_start(cx_sb[:], r128(a2aout[:]))
            re_sb = cp.tile([128, 16, 256], BF16)
            nc.sync.dma_start(re_sb[:], r128(resid.ap()))
            nc.sync.dma_start(k_sb[:], r128(kag[:]))  # prefetch for phase D
            wcq_sb = cp.tile([128, 16, CC], BF16)
            if not uniform:
                mv_sb = cp.tile([128, 256], BF16)
                nc.sync.dma_start(mv_sb[:], mv_in.ap()[:])
                ml_sb = cp.tile([128, 256], BF16)
                nc.sync.dma_start(ml_sb[:], ml_in.ap()[:])
                cxv = cp.tile([128, 16, 256], BF16)
                cxl = cp.tile([128, 16, 256], BF16)
                for kc in range(16):
                    nc.vector.tensor_mul(cxv[:, kc, :], cx_sb[:, kc, :],
                                         mv_sb[:])
                    nc.vector.tensor_mul(cxl[:, kc, :], cx_sb[:, kc, :],
                                         ml_sb[:])
            for mg in range(4):  # stream dense weight in 4 x 2.1MB tiles
                wsrc0 = wde if uniform else wde0
                wde_t = wdp.tile([128, 16, 512], BF16, name=f"wde{mg}",
                                 tag="wde")
                nc.sync.dma_start(
                    wde_t[:], r128(wsrc0.ap()[:, mg * 512:mg * 512 + 512]))
                if not uniform:
                    wde1_t = wdp.tile([128, 16, 512], BF16, name=f"wdeb{mg}",
                                      tag="wdeb")
                    nc.sync.dma_start(
                        wde1_t[:],
                        r128(wde1.ap()[:, mg * 512:mg * 512 + 512]))
                for mi in range(4):
                    mt = mg * 4 + mi
                    ps = dps.tile([128, 256], F32, name="dp", tag="dp")
                    for kc in range(16):
                        nc.tensor.matmul(
                            ps[:], wde_t[:, kc, mi * 128:mi * 128 + 128],
                            (cx_sb if uniform else cxv)[:, kc, :],
                            start=(kc == 0), stop=(kc == 15))
                    if uniform:
                        nc.vector.tensor_add(h1_sb[:, mt, :], ps[:],
                                             re_sb[:, mt, :])
                    else:
                        ps1 = dps.tile([128, 256], F32, name="dp1", tag="dp1")
                        for kc in range(16):
                            nc.tensor.matmul(
                                ps1[:],
                                wde1_t[:, kc, mi * 128:mi * 128 + 128],
                                cxl[:, kc, :],
                                start=(kc == 0), stop=(kc == 15))
                        t_ = scrp.tile([128, 256], F32, name="dt", tag="dt")
                        nc.vector.tensor_add(t_[:], ps[:], ps1[:])
                        nc.vector.tensor_add(h1_sb[:, mt, :], t_[:],
                                             re_sb[:, mt, :])
            nc.sync.dma_start(wcq_sb[:], r128(wcq.ap()))
            # rmsnorm h1 -> h1n (bf16)
            h1n_sb = cp.tile([128, 16, 256], BF16)
            pss = dps.tile([128, 256], F32, name="nps2", tag="dp")
            for kc in range(16):
                sq = scrp.tile([128, 256], BF16, name="sqc", tag="sqc")
                nc.scalar.activation(sq[:], h1_sb[:, kc, :], SQ)
                nc.tensor.matmul(pss[:], ones_bf[:], sq[:],
                                 start=(kc == 0), stop=(kc == 15))
            rms = scrp.tile([128, 256], F32, name="rmsc", tag="rmsc")
            nc.scalar.activation(rms[:], pss[:], SQRT,
                                 scale=1.0 / H, bias=eps_sb[:])
            rinv = scrp.tile([128, 256], F32, name="rinvc", tag="rmsc")
            nc.vector.reciprocal_approx_fast(rinv[:], rms[:])
            for kc in range(16):
                nc.vector.tensor_mul(h1n_sb[:, kc, :], h1_sb[:, kc, :],
                                     rinv[:])
            for mt in range(8):
                ps = dps.tile([128, 256], F32, name="cqp", tag="dp")
                for kc in range(16):
                    nc.tensor.matmul(ps[:],
                                     wcq_sb[:, kc, mt * 128:mt * 128 + 128],
                                     h1n_sb[:, kc, :],
                                     start=(kc == 0), stop=(kc == 15))
                nc.vector.tensor_copy(cq_sb[:, mt, :], ps[:])

        # ===== phase D: cross attention (token-parallel) + cdense =====
        with ExitStack() as pD:
            kp = pD.enter_context(tc.tile_pool(name="kp", bufs=1))
            v_sb2 = kp.tile([128, 16, CC], BF16)
            for r in range(NC_):
                nc.sync.dma_start(
                    v_sb2[:, :, r * 128:r * 128 + 128],
                    vag[r * E:(r + 1) * E, :].rearrange(
                        "(k p) d -> p k d", p=128))
            wcd_sb = kp.tile([128, 8, H], BF16)
            nc.sync.dma_start(wcd_sb[:], r128(wcd.ap()))
            cap = pD.enter_context(tc.tile_pool(name="cap", bufs=3))
            caps = pD.enter_context(tc.tile_pool(name="caps", bufs=2,
                                                 space="PSUM"))
            cacc = pD.enter_context(tc.tile_pool(name="cacc", bufs=1,
                                                 space="PSUM"))
            dits = [(m, kp) for m in range(8) for kp in range(8)]
            dsc_t = {}

            def emit_dsc(idx, dits=dits, dsc_t=dsc_t):
                m, kp = dits[idx]
                # separate PSUM banks per head: concurrent row-group
                # matmuls must not share a bank
                sca = caps.tile([128, 2, 256], F32, name=f"csa{idx}",
                                tag="csa")
                scb = caps.tile([128, 2, 256], F32, name=f"csb{idx}",
                                tag="csb")
                for j, kt in ((0, 2 * kp), (1, 2 * kp + 1)):
                    nc.tensor.matmul(
                        sca[:, j, :],
                        k_sb[0:64, m, kt * 128:kt * 128 + 128],
                        cq_sb[0:64, m, :], start=True, stop=True)
                    nc.tensor.matmul(
                        scb[:, j, :],
                        k_sb[64:128, m, kt * 128:kt * 128 + 128],
                        cq_sb[64:128, m, :], start=True, stop=True)
                dsc_t[idx] = (sca, scb)

            emit_dsc(0)
            psden = pc2 = None
            for idx, (m, kp) in enumerate(dits):
                if kp == 0:
                    # psden slot 0 = head-a denominator, slot 1 = head-b
                    psden = cacc.tile([128, 2, 256], F32, name=f"cps{m}",
                                      tag="cps")
                    pc2 = cacc.tile([128, 256], F32, name=f"cpc{m}",
                                    tag="cpc")
                if idx + 1 < len(dits):
                    emit_dsc(idx + 1)
                sca, scb = dsc_t.pop(idx)
                pra = cap.tile([128, 2, 256], BF16, name=f"cpra{idx}",
                               tag="cpra")
                nc.scalar.activation(pra[:], sca[:], EXP, scale=CSC)
                prb = cap.tile([128, 2, 256], BF16, name=f"cprb{idx}",
                               tag="cprb")
                nc.scalar.activation(prb[:], scb[:], EXP, scale=CSC)
                for j, kt in ((0, 2 * kp), (1, 2 * kp + 1)):
                    nc.tensor.matmul(psden[:, 0, :], ones_bf[:],
                                     pra[:, j, :],
                                     start=(kp == 0 and j == 0),
                                     stop=(kp == 7 and j == 1))
                    nc.tensor.matmul(psden[:, 1, :], ones_bf[:],
                                     prb[:, j, :],
                                     start=(kp == 0 and j == 0),
                                     stop=(kp == 7 and j == 1))
                    nc.tensor.matmul(
                        pc2[0:64, :], v_sb2[:, kt, 128 * m:128 * m + 64],
                        pra[:, j, :], start=(kp == 0 and j == 0),
                        stop=(kp == 7 and j == 1))
                    nc.tensor.matmul(
                        pc2[64:128, :],
                        v_sb2[:, kt, 128 * m + 64:128 * m + 128],
                        prb[:, j, :], start=(kp == 0 and j == 0),
                        stop=(kp == 7 and j == 1))
                if kp == 7:
                    rc = cap.tile([64, 2, 256], F32, name=f"crc{m}",
                                  tag="crc")
                    nc.vector.reciprocal_approx_fast(rc[:], psden[0:64, :, :])
                    nc.vector.tensor_mul(cctx_sb[0:64, m, :], pc2[0:64, :],
                                         rc[:, 0, :])
                    nc.vector.tensor_mul(cctx_sb[64:128, m, :],
                                         pc2[64:128, :], rc[:, 1, :])
            # cdense + residual -> h2, rmsnorm -> h2n -> AG
            d4 = pD.enter_context(tc.tile_pool(name="d4", bufs=1))
            h2_sb = d4.tile([128, 16, 256], F32)
            h2n_sb = d4.tile([128, 16, 256], BF16)
            for mt in range(16):
                ps = dps.tile([128, 256], F32, name="cdp", tag="dp")
                for kc in range(8):
                    nc.tensor.matmul(ps[:],
                                     wcd_sb[:, kc, mt * 128:mt * 128 + 128],
                                     cctx_sb[:, kc, :],
                                     start=(kc == 0), stop=(kc == 7))
                nc.vector.tensor_add(h2_sb[:, mt, :], ps[:],
                                     h1_sb[:, mt, :])
            pss2 = dps.tile([128, 256], F32, name="psd2", tag="dp")
            for kc in range(16):
                sq = scrp.tile([128, 256], BF16, name="sqd2", tag="sqc")
                nc.scalar.activation(sq[:], h2_sb[:, kc, :], SQ)
                nc.tensor.matmul(pss2[:], ones_bf[:], sq[:],
                                 start=(kc == 0), stop=(kc == 15))
            rms2 = scrp.tile([128, 256], F32, name="rmsd2", tag="rmsc")
            nc.scalar.activation(rms2[:], pss2[:], SQRT,
                                 scale=1.0 / H, bias=eps_sb[:])
            rinv2 = scrp.tile([128, 256], F32, name="rinvd", tag="rmsc")
            nc.vector.reciprocal_approx_fast(rinv2[:], rms2[:])
            for kc in range(16):
                nc.vector.tensor_mul(h2n_sb[:, kc, :],
                                     h2_sb[:, kc, :], rinv2[:])
            nc.sync.dma_start(r128(h2nb[:]), h2n_sb[:])
            nc.sync.dma_start(r128(h2out.ap()), h2_sb[:])
        pCD.close()
        nc.gpsimd.collective_compute(
            "AllGather", mybir.AluOpType.bypass, replica_groups=RG,
            ins=[h2nb.opt()], outs=[h2na.opt()])

        # ===== phase F: MLP (routed by expert ranges, bf16) =====
        with ExitStack() as pF:
            fp = pF.enter_context(tc.tile_pool(name="fp", bufs=1))
            hn_sb = fp.tile([128, 16, S], BF16)
            for r in range(NC_):
                nc.sync.dma_start(hn_sb[:, :, r * 256:r * 256 + 256],
                                  r128(h2na[r * H:(r + 1) * H, :]))
            fw = pF.enter_context(tc.tile_pool(name="fw", bufs=1))
            fps = pF.enter_context(tc.tile_pool(name="fps", bufs=2,
                                                space="PSUM"))
            fpd = pF.enter_context(tc.tile_pool(name="fpd", bufs=2,
                                                space="PSUM"))
            fac = pF.enter_context(tc.tile_pool(name="fac", bufs=2))
            fout = pF.enter_context(tc.tile_pool(name="fout", bufs=4))
            for ex, (lo, hi) in ((0, (0, b1)), (1, (b1, S))):
                gsrc = (wgu0, wgu1)[ex]
                dsrc = (wdn0, wdn1)[ex]
                dn_t = fw.tile([128, 6, H], BF16, name=f"dn{ex}", tag="dn")
                nc.sync.dma_start(dn_t[:], r128(dsrc.ap()))
                gwts = []
                for pi in range(6):
                    gw = 128 if pi < 5 else 48
                    gwt = fw.tile([128, 16, 256], BF16,
                                  name=f"guw{ex}{pi}", tag=f"guw{pi}")
                    nc.sync.dma_start(
                        gwt[:, :, :2 * gw],
                        r128(gsrc.ap()[:, pi * 256:pi * 256 + 2 * gw]))
                    gwts.append(gwt)
                for a0 in range(0, S, 512):
                    c0, c1 = max(a0, lo), min(a0 + 512, hi)
                    if c0 >= c1:
                        continue
                    t0_, W = a0, 512
                    eo, ew = c0 - a0, c1 - c0
                    act = fac.tile([128, 6, 512], BF16, name="act", tag="act")
                    for pi in range(6):
                        gw = 128 if pi < 5 else 48
                        gwt = gwts[pi]
                        pg = fps.tile([128, 512], F32, name="pg", tag="pg")
                        pu = fps.tile([128, 512], F32, name="pu", tag="pu")
                        for kc in range(16):
                            nc.tensor.matmul(pg[:gw, :W], gwt[:, kc, :gw],
                                             hn_sb[:, kc, t0_:t0_ + 512],
                                             start=(kc == 0), stop=(kc == 15))
                            nc.tensor.matmul(pu[:gw, :W], gwt[:, kc, gw:2 * gw],
                                             hn_sb[:, kc, t0_:t0_ + 512],
                                             start=(kc == 0), stop=(kc == 15))
                        gs = scrp.tile([128, 512], F32, name="gs", tag="gs")
                        nc.scalar.activation(gs[:gw, :W], pg[:gw, :W], SILU)
                        nc.vector.tensor_mul(act[:gw, pi, :W],
                                             gs[:gw, :W], pu[:gw, :W])
                    for mt in range(16):
                        pd = fpd.tile([128, 512], F32, name="pd", tag="pd")
                        for pi in range(6):
                            kw = 128 if pi < 5 else 48
                            nc.tensor.matmul(
                                pd[:, :W],
                                dn_t[:kw, pi, mt * 128:mt * 128 + 128],
                                act[:kw, pi, :W],
                                start=(pi == 0), stop=(pi == 5))
                        ot = fout.tile([128, 512], F32, name="fot", tag="fot")
                        if mt % 2 == 0:
                            nc.vector.tensor_copy(ot[:, eo:eo + ew],
                                                  pd[:, eo:eo + ew])
                        else:
                            nc.scalar.copy(ot[:, eo:eo + ew],
                                           pd[:, eo:eo + ew])
                        nc.sync.dma_start(
                            y.ap()[mt * 128:mt * 128 + 128, c0:c1],
                            ot[:, eo:eo + ew])
    nc.compile()
    return nc


_CACHE = {}


def kernel(**inputs):
    import ml_dtypes
    vm = np.asarray(inputs["vision_token_ids"]).astype(bool)
    lm = np.asarray(inputs["language_token_ids"]).astype(bool)
    g0 = np.where(vm & ~lm)[0]; g1 = np.where(vm & lm)[0]
    g2 = np.where(~vm & lm)[0]; g3 = np.where(~vm & ~lm)[0]
    perm = np.concatenate([g0, g1, g2, g3])
    b0 = len(g0); b1 = b0 + len(g1); b2 = b1 + len(g2)

    f32 = lambda x: np.ascontiguousarray(np.asarray(x, np.float32))
    bf = lambda x: np.ascontiguousarray(np.asarray(x).astype(ml_dtypes.bfloat16))
    pos = np.asarray(inputs["positions"]).astype(np.float32)
    half = HD // 2
    inv_freq = 1.0 / (ROPE_BASE ** (np.arange(half, dtype=np.float32) / half))
    fr = pos[:, None] * inv_freq[None, :]
    cos2 = np.concatenate([np.cos(fr)] * 2, 1).T[:, perm]
    sin2 = np.concatenate([np.sin(fr)] * 2, 1).T[:, perm]
    rot = np.zeros((HD, HD), np.float32)
    rot[np.arange(half), np.arange(half) + half] = -1.0
    rot[np.arange(half) + half, np.arange(half)] = 1.0
    op = np.asarray(inputs["positions"])[perm]

    # causal block states + diagonal multiplicative masks
    blk = []
    mrows = []
    for ci in range(4):
        qv = op[512 * ci:512 * ci + 512]
        for kt in range(16):
            kv = op[128 * kt:128 * kt + 128]
            if kv.max() <= qv.min():
                blk.append(1)
            elif kv.min() > qv.max():
                blk.append(0)
            else:
                blk.append(2)
                mrows.append((qv[None, :] >= kv[:, None]).astype(np.float32))
    blk = tuple(blk)
    maskd = (np.concatenate(mrows, 0) if mrows
             else np.zeros((128, 512), np.float32))

    # per-chunk expert combos (0=vis, 1=both, 2=lang, 3=neither)
    combo = np.full(S, 3, np.int8)
    combo[:b0] = 0; combo[b0:b1] = 1; combo[b1:b2] = 2
    chunk_combo = []
    uniform = True
    for j in range(NC_):
        cj = combo[256 * j:256 * j + 256]
        if (cj == cj[0]).all():
            chunk_combo.append(int(cj[0]))
        else:
            chunk_combo.append(-1)
            uniform = False

    wln_in = f32(inputs["w_ln_in"])[:, None]
    wln_pa = f32(inputs["w_ln_post_attn"])[:, None]
    wln_pc = f32(inputs["w_ln_post_cross"])[:, None]
    wqkv = [f32(inputs["w_vis_qkv"]) * wln_in, f32(inputs["w_lang_qkv"]) * wln_in]
    wd = [f32(inputs["w_vis_dense"]), f32(inputs["w_lang_dense"])]
    wgu = [f32(inputs["w_vis_gate_up"]) * wln_pc,
           f32(inputs["w_lang_gate_up"]) * wln_pc]
    wdn = [f32(inputs["w_vis_down"]), f32(inputs["w_lang_down"])]
    wkvf = f32(inputs["w_cross_kv"])
    hTp = f32(inputs["hidden_states"]).T[:, perm].copy()

    def interleave(w):  # w [H, 2*IS] = [gate | up]
        cols = []
        for i in range(5):
            cols.append(w[:, 128 * i:128 * i + 128])
            cols.append(w[:, IS + 128 * i:IS + 128 * i + 128])
        cols.append(w[:, 640:IS]); cols.append(w[:, IS + 640:2 * IS])
        return np.ascontiguousarray(np.concatenate(cols, 1))

    key = (b0, b1, b2, blk, uniform)
    if key not in _CACHE:
        _CACHE.clear()
        _CACHE[key] = build_kernel(b0, b1, b2, blk, uniform)
    nc = _CACHE[key]

    # dense weight combos (bf16, built once per distinct combo)
    wde_by_combo = {}
    for cb in set(chunk_combo):
        if cb == 0:
            wde_by_combo[cb] = bf(wd[0])
        elif cb == 1:
            wde_by_combo[cb] = bf(wd[0] + wd[1])
        elif cb == 2:
            wde_by_combo[cb] = bf(wd[1])
        elif cb == 3:
            wde_by_combo[cb] = bf(np.zeros_like(wd[0]))

    in_maps = []
    for c in range(NC_):
        qs = slice(256 * c, 256 * c + 256)
        m = dict(
            hT=bf(hTp),
            resid=bf(hTp[:, qs]),
            wqkv0=bf(np.concatenate([wqkv[0][:, qs], wqkv[0][:, H:][:, qs],
                                     wqkv[0][:, 2 * H:][:, qs]], 1)),
            wqkv1=bf(np.concatenate([wqkv[1][:, qs], wqkv[1][:, H:][:, qs],
                                     wqkv[1][:, 2 * H:][:, qs]], 1)),
            cos2=bf(cos2), sin2=bf(sin2), rotT=bf(rot.T),
            onesb=np.ones((128, 128), ml_dtypes.bfloat16),
            maskd=bf(maskd),
            encT=bf(f32(inputs["encoder_embeds"]).T),
            wkc=bf(wkvf[:, 128 * c:128 * c + 128]),
            wvc=bf(wkvf[:, CC + 128 * c:CC + 128 * c + 128]),
            wcq=bf(f32(inputs["w_cross_q"]) * wln_pa),
            wcd=bf(f32(inputs["w_cross_dense"])),
            wgu0=bf(interleave(np.concatenate(
                [wgu[0][:, IS * c:IS * c + IS],
                 wgu[0][:, I + IS * c:I + IS * c + IS]], 1))),
            wgu1=bf(interleave(np.concatenate(
                [wgu[1][:, IS * c:IS * c + IS],
                 wgu[1][:, I + IS * c:I + IS * c + IS]], 1))),
            wdn0=bf(np.concatenate([wdn[0][IS * c:IS * c + IS],
                                    np.zeros((ISP - IS, H), np.float32)], 0)),
            wdn1=bf(np.concatenate([wdn[1][IS * c:IS * c + IS],
                                    np.zeros((ISP - IS, H), np.float32)], 0)),
        )
        if uniform:
            m["wde"] = wde_by_combo[chunk_combo[c]]
        else:
            m["wde0"] = bf(wd[0])
            m["wde1"] = bf(wd[1])
            pv = vm[perm][qs].astype(np.float32)
            pl = lm[perm][qs].astype(np.float32)
            m["mv"] = bf(np.broadcast_to(pv[None, :], (128, 256)).copy())
            m["ml"] = bf(np.broadcast_to(pl[None, :], (128, 256)).copy())
        in_maps.append(m)

    # wqkv slot layout check: slots are [q(2x128) | k(2x128) | v(2x128)]
    # per-core head pair -> columns 128c..128c+256 of each of q,k,v.

    if os.environ.get("KSIM"):
        from concourse.bass_interp import MultiCoreSim
        sim = MultiCoreSim(nc, num_cores=NC_)
        for c, cs in sim.cores.items():
            for name, val in in_maps[c].items():
                cs.tensor(name)[:] = val
        sim.simulate(check_with_hw=False)
        results = [dict(y=np.array(sim.cores[c].tensor("y")),
                        h2out=np.array(sim.cores[c].tensor("h2out")))
                   for c in range(NC_)]
        kernel.last_exec_ns = 0
    else:
        trace = bool(int(os.environ.get("KTRACE", "0")))
        res = run_bass_kernel_spmd(nc, in_maps, core_ids=list(range(NC_)),
                                   trace=trace)
        kernel.last_exec_ns = res.exec_time_ns
        results = res.results
    tot = results[0]["y"].astype(np.float64)
    for c in range(1, NC_):
        tot += results[c]["y"]
    for c in range(NC_):
        tot[:, 256 * c:256 * c + 256] += results[c]["h2out"]
    out = np.empty((S, H), np.float32)
    out[perm, :] = tot.T.astype(np.float32)
    return out


# revision 32
# speedup vs baseline: 1.2487x; 1.0471x over previous
"""Trainium2 Bass kernel for nn_CogAgentDecoderLayer (8-core SPMD).

Feature-major activations [feat, tok] in permuted token order
(vis-only | both | lang-only | neither). TP: QKV/self-attn by heads
(2/core), MLP by intermediate slice (688/core). Cross-attn K/V computed
sharded (128 of 1024 dims per core) + AllGathered early, overlapped with
self-attention. Self-attn ctx redistributed with AllToAll (head-shards ->
token-shards), then dense/cross-attn/cdense run token-parallel (256
tok/core). Final MLP partial sums reduced on host. Self-attention skips
fully-masked causal blocks; diagonal blocks use multiplicative 0/1 masks.
bf16 matmuls throughout, fp32 psum/residual/norm stats.
"""
import os
import sys
import types
import numpy as np
from contextlib import ExitStack

# concourse's trace path does `from antenv.axon_hooks import ...`; provide a
# stub registry if the module is missing so tracing degrades instead of
# crashing (the boot hook registers itself here when available).
try:
    import antenv.axon_hooks  # noqa: F401
except Exception:
    try:
        import antenv
        _m = types.ModuleType("antenv.axon_hooks")
        _m._hook = None
        _m.set_axon_ntff_profile_hook = lambda h: setattr(_m, "_hook", h)
        _m.get_axon_ntff_profile_hook = lambda: _m._hook
        sys.modules["antenv.axon_hooks"] = _m
        antenv.axon_hooks = _m
    except Exception:
        pass

from concourse import bacc, tile, mybir
from concourse.bass_utils import run_bass_kernel_spmd

NC_ = 8
S, E, H, NH, HD = 2048, 2048, 2048, 16, 128
CH, CC, CHD = 1024, 1024, 64
I = 5504
IS = I // NC_          # 688
ISP = 768              # padded to 6*128
EPS = 1e-5
ROPE_BASE = 10000.0
F32 = mybir.dt.float32
F32R = mybir.dt.float32r
BF16 = mybir.dt.bfloat16


def _segs(lo, hi, b0, b1, b2):
    pts = sorted({lo, hi, *[b for b in (b0, b1, b2) if lo < b < hi]})
    out = []
    for s, e in zip(pts, pts[1:]):
        ex = []
        if s < b1:
            ex.append(0)
        if b0 <= s < b2:
            ex.append(1)
        out.append((s, e, ex))
    return out


def _chunks(lo, hi, w):
    out = []
    while lo < hi:
        out.append((lo, min(lo + w, hi)))
        lo += w
    return out


def build_kernel(b0, b1, b2, blk, uniform):
    """blk: 64-tuple, state per (ci, kt): 0=skip, 1=visible, 2=partial.
    uniform: every 256-token chunk has a single expert-combo (host packs
    the right dense weight per core)."""
    nc = bacc.Bacc("TRN2", target_bir_lowering=False, debug=False,
                   num_devices=NC_)
    din = lambda n, sh, dt: nc.dram_tensor(n, sh, dt, kind="ExternalInput")
    hT = din("hT", [H, S], BF16)
    resid = din("resid", [H, 256], BF16)
    wqkv0 = din("wqkv0", [H, 768], BF16)
    wqkv1 = din("wqkv1", [H, 768], BF16)
    cos2 = din("cos2", [128, S], BF16)
    sin2 = din("sin2", [128, S], BF16)
    rotT = din("rotT", [128, 128], BF16)
    onesb = din("onesb", [128, 128], BF16)
    nmask = max(1, sum(1 for st in blk if st == 2))
    maskd = din("maskd", [128 * nmask, 512], BF16)
    encT = din("encT", [CH, E], BF16)
    wkc = din("wkc", [CH, 128], BF16)
    wvc = din("wvc", [CH, 128], BF16)
    if uniform:
        wde = din("wde", [H, H], BF16)
    else:
        wde0 = din("wde0", [H, H], BF16)
        wde1 = din("wde1", [H, H], BF16)
        mv_in = din("mv", [128, 256], BF16)
        ml_in = din("ml", [128, 256], BF16)
    wcq = din("wcq", [H, CC], BF16)
    wcd = din("wcd", [CC, H], BF16)
    wgu0 = din("wgu0", [H, 2 * IS], BF16)
    wgu1 = din("wgu1", [H, 2 * IS], BF16)
    wdn0 = din("wdn0", [ISP, H], BF16)
    wdn1 = din("wdn1", [ISP, H], BF16)
    y = nc.dram_tensor("y", [H, S], F32, kind="ExternalOutput")
    h2out = nc.dram_tensor("h2out", [H, 256], F32, kind="ExternalOutput")

    SC = 1.0 / float(np.sqrt(HD))
    CSC = 1.0 / float(np.sqrt(CHD))
    EXP = mybir.ActivationFunctionType.Exp
    SQ = mybir.ActivationFunctionType.Square
    SQRT = mybir.ActivationFunctionType.Sqrt
    SILU = mybir.ActivationFunctionType.Silu
    r128 = lambda ap: ap.rearrange("(c p) n -> p c n", p=128)
    RG = [list(range(NC_))]

    with tile.TileContext(nc) as tc, ExitStack() as top:
        const = top.enter_context(tc.tile_pool(name="const", bufs=1))
        ones_bf = const.tile([128, 128], BF16)
        nc.sync.dma_start(ones_bf[:], onesb.ap()[:])
        rot_sb = const.tile([128, 128], BF16)
        nc.sync.dma_start(rot_sb[:], rotT.ap()[:])
        from concourse.masks import make_identity
        ident = const.tile([128, 128], BF16)
        make_identity(nc, ident[:])
        cos_sb = const.tile([128, S], BF16)
        sin_sb = const.tile([128, S], BF16)
        eps_sb = const.tile([128, 1], F32)
        nc.vector.memset(eps_sb[:], EPS)

        dram = top.enter_context(tc.tile_pool(name="dram", bufs=1, space="DRAM"))
        kbnc = dram.tile([128, E], BF16)
        vbnc = dram.tile([E, 128], BF16)
        kag = dram.tile([NC_ * 128, E], BF16, addr_space="Shared")
        vag = dram.tile([NC_ * E, 128], BF16, addr_space="Shared")
        a2ain = dram.tile([H, 256], BF16)
        a2aout = dram.tile([H, 256], BF16)
        h2nb = dram.tile([H, 256], BF16)
        h2na = dram.tile([NC_ * H, 256], BF16, addr_space="Shared")

        scrp = top.enter_context(tc.tile_pool(name="scr", bufs=2))

        # tiles for A/B; h DMA deferred until after phase-0 inputs so the
        # cross-KV compute (first PE work) isn't starved by the h transfer
        pAB = top.enter_context(ExitStack())
        qkp = pAB.enter_context(tc.tile_pool(name="qkp", bufs=1))
        qkv_sb = qkp.tile([128, 6, S], BF16)      # q0 q1 k0 k1 v0 v1
        v_sb = qkp.tile([128, 16, 256], BF16)     # token-major v
        ctx_sb = qkp.tile([128, 2, S], BF16)
        hps = top.enter_context(ExitStack())
        hp = hps.enter_context(tc.tile_pool(name="hp", bufs=1))
        h_sb = hp.tile([128, 16, S], BF16)

        # ===== phase 0: cross K/V shard compute + AllGathers =====
        with ExitStack() as p0:
            ep = p0.enter_context(tc.tile_pool(name="ep", bufs=1))
            enc_sb = ep.tile([128, 8, E], BF16)
            nc.sync.dma_start(enc_sb[:], r128(encT.ap()))
            wk_sb = ep.tile([128, 8, 128], BF16)
            nc.sync.dma_start(wk_sb[:], r128(wkc.ap()))
            wv_sb = ep.tile([128, 8, 128], BF16)
            nc.sync.dma_start(wv_sb[:], r128(wvc.ap()))
            kb_sb = ep.tile([128, E], BF16)
            vb_sb = ep.tile([128, 16, 128], BF16)
            kvp = p0.enter_context(tc.tile_pool(name="kvp", bufs=2,
                                                space="PSUM"))
            for n0, n1 in _chunks(0, E, 512):
                ps = kvp.tile([128, 512], F32, name="kps", tag="kps")
                for kc in range(8):
                    nc.tensor.matmul(ps[:], wk_sb[:, kc, :],
                                     enc_sb[:, kc, n0:n1],
                                     start=(kc == 0), stop=(kc == 7))
                nc.vector.tensor_copy(kb_sb[:, n0:n1], ps[:])
            for kt in range(16):
                ps = kvp.tile([128, 128], F32, name="vps", tag="vps")
                for kc in range(8):
                    nc.tensor.matmul(ps[:],
                                     enc_sb[:, kc, kt * 128:kt * 128 + 128],
                                     wv_sb[:, kc, :],
                                     start=(kc == 0), stop=(kc == 7))
                nc.vector.tensor_copy(vb_sb[:, kt, :], ps[:])
            nc.sync.dma_start(kbnc[:], kb_sb[:])
            nc.sync.dma_start(vbnc[:].rearrange("(k p) d -> p k d", p=128),
                              vb_sb[:])
        nc.gpsimd.collective_compute(
            "AllGather", mybir.AluOpType.bypass, replica_groups=RG,
            ins=[kbnc.opt()], outs=[kag.opt()])
        nc.gpsimd.collective_compute(
            "AllGather", mybir.AluOpType.bypass, replica_groups=RG,
            ins=[vbnc.opt()], outs=[vag.opt()])

        # ===== phase A: per-chunk rmsnorm + QKV + rope + vT =====
        with ExitStack() as pA2:
            wp = pA2.enter_context(tc.tile_pool(name="wp", bufs=1))
            wq_sb = [wp.tile([128, 16, 768], BF16, name=f"wqa{x}",
                             tag=f"wqa{x}") for x in range(2)]
            # DMA order tuned for earliest first matmul: h chunk 0, then
            # QKV weights, then the rest of h, then rope tables
            nc.sync.dma_start(
                h_sb[:, :, 0:512],
                hT.ap()[:, 0:512].rearrange("(c p) n -> p c n", p=128))
            nc.sync.dma_start(wq_sb[0][:], r128(wqkv0.ap()))
            nc.sync.dma_start(wq_sb[1][:], r128(wqkv1.ap()))
            for t0, t1 in _chunks(512, S, 512):
                nc.sync.dma_start(
                    h_sb[:, :, t0:t1],
                    hT.ap()[:, t0:t1].rearrange("(c p) n -> p c n", p=128))
            nc.sync.dma_start(cos_sb[:], cos2.ap()[:])
            nc.sync.dma_start(sin_sb[:], sin2.ap()[:])
            nrm = pA2.enter_context(tc.tile_pool(name="nrm", bufs=2))
            mps = pA2.enter_context(tc.tile_pool(name="mps", bufs=2,
                                                 space="PSUM"))
            for ci, (t0, t1) in enumerate(_chunks(0, S, 512)):
                pss = mps.tile([128, 512], F32, name="pss", tag="qps1")
                for kc in range(16):
                    sq = nrm.tile([128, 512], BF16, name="sq", tag="sq")
                    nc.scalar.activation(sq[:], h_sb[:, kc, t0:t1], SQ)
                    nc.tensor.matmul(pss[:], ones_bf[:], sq[:],
                                     start=(kc == 0), stop=(kc == 15))
                rms = nrm.tile([128, 512], F32, name="rms", tag="rms")
                nc.scalar.activation(rms[:], pss[:], SQRT,
                                     scale=1.0 / H, bias=eps_sb[:])
                rinv = nrm.tile([128, 512], F32, name="rinv", tag="rinv")
                nc.vector.reciprocal_approx_fast(rinv[:], rms[:])
                rinvb = nrm.tile([128, 512], BF16, name="rinvb", tag="rinvb")
                nc.vector.tensor_copy(rinvb[:], rinv[:])
                for kc in range(16):
                    nc.vector.tensor_mul(h_sb[:, kc, t0:t1],
                                         h_sb[:, kc, t0:t1], rinvb[:])
                sg = [x for x in _segs(t0, t1, b0, b1, b2) if x[2]]
                for slot in range(6):
                    if sg:
                        need = sorted({x for _, _, ex in sg for x in ex})
                        pss_ = {}
                        for x in need:
                            ps = mps.tile([128, 512], F32, name=f"qps{x}",
                                          tag=f"qps{x}")
                            for kc in range(16):
                                nc.tensor.matmul(
                                    ps[:],
                                    wq_sb[x][:, kc, slot * 128:slot * 128 + 128],
                                    h_sb[:, kc, t0:t1],
                                    start=(kc == 0), stop=(kc == 15))
                            pss_[x] = ps
                        for s, e, ex in sg:
                            if len(ex) == 1:
                                nc.vector.tensor_copy(
                                    qkv_sb[:, slot, s:e],
                                    pss_[ex[0]][:, s - t0:e - t0])
                            else:
                                nc.vector.tensor_add(qkv_sb[:, slot, s:e],
                                                     pss_[0][:, s - t0:e - t0],
                                                     pss_[1][:, s - t0:e - t0])
                    if t1 > b2 > t0:
                        nc.vector.memset(qkv_sb[:, slot, b2:t1], 0.0)
                    elif t0 >= b2:
                        nc.vector.memset(qkv_sb[:, slot, t0:t1], 0.0)
                # rope on q,k of this chunk
                for slot in range(4):
                    rp = mps.tile([128, 512], F32, name="rps", tag="qps0")
                    nc.tensor.matmul(rp[:], rot_sb[:],
                                     qkv_sb[:, slot, t0:t1],
                                     start=True, stop=True)
                    rpb = scrp.tile([128, 512], BF16, name="rpb", tag="rpb")
                    nc.scalar.copy(rpb[:], rp[:])
                    c1 = scrp.tile([128, 512], BF16, name="ropec", tag="ropec")
                    nc.vector.tensor_mul(c1[:], qkv_sb[:, slot, t0:t1],
                                         cos_sb[:, t0:t1])
                    nc.vector.tensor_mul(rpb[:], rpb[:], sin_sb[:, t0:t1])
                    nc.vector.tensor_add(qkv_sb[:, slot, t0:t1],
                                         c1[:], rpb[:])
                # v -> token-major via PE transpose (this chunk's tokens)
                for hh in range(2):
                    for tt in range(t0 // 128, t1 // 128):
                        tp = mps.tile([128, 128], BF16, name="tps", tag="tps")
                        nc.tensor.transpose(
                            tp[:],
                            qkv_sb[:, 4 + hh, tt * 128:tt * 128 + 128],
                            ident[:])
                        nc.vector.tensor_copy(
                            v_sb[:, tt, hh * 128:hh * 128 + 128], tp[:])
        hps.close()  # h_sb no longer needed; frees 8.4MB for prefetch

        # ===== phase B: self-attention (causal skip, mult. diag masks) =====
        mrow = {}
        _mr = 0
        for ci in range(4):
            for kt in range(16):
                if blk[ci * 16 + kt] == 2:
                    mrow[(ci, kt)] = _mr
                    _mr += 1
        with ExitStack() as pB:
            ap_ = pB.enter_context(tc.tile_pool(name="ap", bufs=4))
            aps = pB.enter_context(tc.tile_pool(name="aps", bufs=3,
                                                space="PSUM"))
            accp = pB.enter_context(tc.tile_pool(name="accp", bufs=1,
                                                 space="PSUM"))
            for ci, (t0, t1) in enumerate(_chunks(0, S, 512)):
                live = [(kt, blk[ci * 16 + kt]) for kt in range(16)
                        if blk[ci * 16 + kt] != 0]
                pss_ = [accp.tile([128, 512], F32, name=f"pbs{h}",
                                  tag=f"pbs{h}") for h in range(2)]
                psc_ = [accp.tile([128, 512], F32, name=f"pbc{h}",
                                  tag=f"pbc{h}") for h in range(2)]
                its = [(kt, st, hh) for kt, st in live for hh in range(2)]
                sc_t, mt_t = {}, {}

                def emit_sc(i, ci=ci, t0=t0, t1=t1, its=its, sc_t=sc_t,
                            mt_t=mt_t):
                    kt, st, hh = its[i]
                    if st == 2 and hh == 0:
                        mt_ = ap_.tile([128, 512], BF16, name="mt", tag="mt")
                        r0 = mrow[(ci, kt)] * 128
                        nc.sync.dma_start(mt_[:], maskd.ap()[r0:r0 + 128, :])
                        mt_t[kt] = mt_
                    sc = aps.tile([128, 512], F32, name=f"sc{ci}_{i}",
                                  tag="sc")
                    nc.tensor.matmul(
                        sc[:], qkv_sb[:, 2 + hh, kt * 128:kt * 128 + 128],
                        qkv_sb[:, hh, t0:t1], start=True, stop=True)
                    sc_t[i] = sc

                emit_sc(0)
                n_it = len(its)
                if n_it > 1:
                    emit_sc(1)
                for i, (kt, st, hh) in enumerate(its):
                    if i + 2 < n_it:
                        emit_sc(i + 2)
                    sc = sc_t.pop(i)
                    pr = ap_.tile([128, 512], BF16, name="pr", tag="pr")
                    nc.scalar.activation(pr[:], sc[:], EXP, scale=SC)
                    if st == 2:
                        nc.vector.tensor_mul(pr[:], pr[:], mt_t[kt])
                    nc.tensor.matmul(pss_[hh][:], ones_bf[:], pr[:],
                                     start=(i < 2), stop=(i >= n_it - 2))
                    nc.tensor.matmul(
                        psc_[hh][:],
                        v_sb[:, kt, hh * 128:hh * 128 + 128],
                        pr[:], start=(i < 2), stop=(i >= n_it - 2))
                for hh in range(2):
                    rc = ap_.tile([128, 512], F32, name="rc", tag="rc")
                    nc.vector.reciprocal_approx_fast(rc[:], pss_[hh][:])
                    nc.vector.tensor_mul(ctx_sb[:, hh, t0:t1],
                                         psc_[hh][:], rc[:])
                for hh in range(2):
                    for j in (2 * ci, 2 * ci + 1):
                        nc.sync.dma_start(
                            a2ain[256 * j + 128 * hh:
                                  256 * j + 128 * hh + 128, :],
                            ctx_sb[:, hh, 256 * j:256 * j + 256])
        pAB.close()
        nc.gpsimd.collective_compute(
            "AllToAll", mybir.AluOpType.bypass, replica_groups=RG,
            ins=[a2ain.opt()], outs=[a2aout.opt()])

        # ===== phase C: dense (token-parallel) + h1 + rmsnorm + cq =====
        pCD = top.enter_context(ExitStack())
        cd = pCD.enter_context(tc.tile_pool(name="cd", bufs=1))
        h1_sb = cd.tile([128, 16, 256], F32)
        cq_sb = cd.tile([128, 8, 256], BF16)
        cctx_sb = cd.tile([128, 8, 256], BF16)
        dps = pCD.enter_context(tc.tile_pool(name="dps", bufs=2,
                                             space="PSUM"))
        kpre = pCD.enter_context(tc.tile_pool(name="kpre", bufs=1))
        k_sb = kpre.tile([128, 8, E], BF16)
        with ExitStack() as pC:
            cp = pC.enter_context(tc.tile_pool(name="cp", bufs=1))
            wdp = pC.enter_context(tc.tile_pool(name="wdp", bufs=2))
            cx_sb = cp.tile([128, 16, 256], BF16)
            nc.sync.dma_start(cx_sb[:], r128(a2aout[:]))
            re_sb = cp.tile([128, 16, 256], BF16)
            nc.sync.dma_start(re_sb[:], r128(resid.ap()))
            nc.sync.dma_start(k_sb[:], r128(kag[:]))  # prefetch for phase D
            wcq_sb = cp.tile([128, 16, CC], BF16)
            nc.sync.dma_start(wcq_sb[:], r128(wcq.ap()))
            if not uniform:
                mv_sb = cp.tile([128, 256], BF16)
                nc.sync.dma_start(mv_sb[:], mv_in.ap()[:])
                ml_sb = cp.tile([128, 256], BF16)
                nc.sync.dma_start(ml_sb[:], ml_in.ap()[:])
                cxv = cp.tile([128, 16, 256], BF16)
                cxl = cp.tile([128, 16, 256], BF16)
                for kc in range(16):
                    nc.vector.tensor_mul(cxv[:, kc, :], cx_sb[:, kc, :],
                                         mv_sb[:])
                    nc.vector.tensor_mul(cxl[:, kc, :], cx_sb[:, kc, :],
                                         ml_sb[:])
            for mg in range(4):  # stream dense weight in 4 x 2.1MB tiles
                wsrc0 = wde if uniform else wde0
                wde_t = wdp.tile([128, 16, 512], BF16, name=f"wde{mg}",
                                 tag="wde")
                nc.sync.dma_start(
                    wde_t[:], r128(wsrc0.ap()[:, mg * 512:mg * 512 + 512]))
                if not uniform:
                    wde1_t = wdp.tile([128, 16, 512], BF16, name=f"wdeb{mg}",
                                      tag="wdeb")
                    nc.sync.dma_start(
                        wde1_t[:],
                        r128(wde1.ap()[:, mg * 512:mg * 512 + 512]))
                for mi in range(4):
                    mt = mg * 4 + mi
                    ps = dps.tile([128, 256], F32, name="dp", tag="dp")
                    for kc in range(16):
                        nc.tensor.matmul(
                            ps[:], wde_t[:, kc, mi * 128:mi * 128 + 128],
                            (cx_sb if uniform else cxv)[:, kc, :],
                            start=(kc == 0), stop=(kc == 15))
                    if uniform:
                        nc.vector.tensor_add(h1_sb[:, mt, :], ps[:],
                                             re_sb[:, mt, :])
                    else:
                        ps1 = dps.tile([128, 256], F32, name="dp1", tag="dp1")
                        for kc in range(16):
                            nc.tensor.matmul(
                                ps1[:],
                                wde1_t[:, kc, mi * 128:mi * 128 + 128],
                                cxl[:, kc, :],
                                start=(kc == 0), stop=(kc == 15))
                        t_ = scrp.tile([128, 256], F32, name="dt", tag="dt")
                        nc.vector.tensor_add(t_[:], ps[:], ps1[:])
                        nc.vector.tensor_add(h1_sb[:, mt, :], t_[:],
                                             re_sb[:, mt, :])
            # rmsnorm h1 -> h1n (bf16)
            h1n_sb = cp.tile([128, 16, 256], BF16)
            pss = dps.tile([128, 256], F32, name="nps2", tag="dp")
            for kc in range(16):
                sq = scrp.tile([128, 256], BF16, name="sqc", tag="sqc")
                nc.scalar.activation(sq[:], h1_sb[:, kc, :], SQ)
                nc.tensor.matmul(pss[:], ones_bf[:], sq[:],
                                 start=(kc == 0), stop=(kc == 15))
            rms = scrp.tile([128, 256], F32, name="rmsc", tag="rmsc")
            nc.scalar.activation(rms[:], pss[:], SQRT,
                                 scale=1.0 / H, bias=eps_sb[:])
            rinv = scrp.tile([128, 256], F32, name="rinvc", tag="rmsc")
            nc.vector.reciprocal_approx_fast(rinv[:], rms[:])
            for kc in range(16):
                nc.vector.tensor_mul(h1n_sb[:, kc, :], h1_sb[:, kc, :],
                                     rinv[:])
            for mt in range(8):
                ps = dps.tile([128, 256], F32, name="cqp", tag="dp")
                for kc in range(16):
                    nc.tensor.matmul(ps[:],
                                     wcq_sb[:, kc, mt * 128:mt * 128 + 128],
                                     h1n_sb[:, kc, :],
                                     start=(kc == 0), stop=(kc == 15))
                nc.vector.tensor_copy(cq_sb[:, mt, :], ps[:])

        # ===== phase D: cross attention (token-parallel) + cdense =====
        with ExitStack() as pD:
            kp = pD.enter_context(tc.tile_pool(name="kp", bufs=1))
            v_sb2 = kp.tile([128, 16, CC], BF16)
            for r in range(NC_):
                nc.sync.dma_start(
                    v_sb2[:, :, r * 128:r * 128 + 128],
                    vag[r * E:(r + 1) * E, :].rearrange(
                        "(k p) d -> p k d", p=128))
            wcd_sb = kp.tile([128, 8, H], BF16)
            nc.sync.dma_start(wcd_sb[:], r128(wcd.ap()))
            cap = pD.enter_context(tc.tile_pool(name="cap", bufs=4))
            caps = pD.enter_context(tc.tile_pool(name="caps", bufs=2,
                                                 space="PSUM"))
            cacc = pD.enter_context(tc.tile_pool(name="cacc", bufs=1,
                                                 space="PSUM"))
            dits = [(m, kp) for m in range(8) for kp in range(8)]
            dsc_t = {}

            def emit_dsc(idx, dits=dits, dsc_t=dsc_t):
                m, kp = dits[idx]
                # separate PSUM banks per head: concurrent row-group
                # matmuls must not share a bank
                sca = caps.tile([128, 2, 256], F32, name=f"csa{idx}",
                                tag="csa")
                scb = caps.tile([128, 2, 256], F32, name=f"csb{idx}",
                                tag="csb")
                for j, kt in ((0, 2 * kp), (1, 2 * kp + 1)):
                    nc.tensor.matmul(
                        sca[:, j, :],
                        k_sb[0:64, m, kt * 128:kt * 128 + 128],
                        cq_sb[0:64, m, :], start=True, stop=True)
                    nc.tensor.matmul(
                        scb[:, j, :],
                        k_sb[64:128, m, kt * 128:kt * 128 + 128],
                        cq_sb[64:128, m, :], start=True, stop=True)
                dsc_t[idx] = (sca, scb)

            emit_dsc(0)
            psden = pc2 = None
            for idx, (m, kp) in enumerate(dits):
                if kp == 0:
                    # psden slot 0 = head-a denominator, slot 1 = head-b
                    psden = cacc.tile([128, 2, 256], F32, name=f"cps{m}",
                                      tag="cps")
                    pc2 = cacc.tile([128, 256], F32, name=f"cpc{m}",
                                    tag="cpc")
                if idx + 1 < len(dits):
                    emit_dsc(idx + 1)
                sca, scb = dsc_t.pop(idx)
                pra = cap.tile([128, 2, 256], BF16, name=f"cpra{idx}",
                               tag="cpra")
                nc.scalar.activation(pra[:], sca[:], EXP, scale=CSC)
                prb = cap.tile([128, 2, 256], BF16, name=f"cprb{idx}",
                               tag="cprb")
                nc.scalar.activation(prb[:], scb[:], EXP, scale=CSC)
                for j, kt in ((0, 2 * kp), (1, 2 * kp + 1)):
                    nc.tensor.matmul(psden[:, 0, :], ones_bf[:],
                                     pra[:, j, :],
                                     start=(kp == 0 and j == 0),
                                     stop=(kp == 7 and j == 1))
                    nc.tensor.matmul(psden[:, 1, :], ones_bf[:],
                                     prb[:, j, :],
                                     start=(kp == 0 and j == 0),
                                     stop=(kp == 7 and j == 1))
                    nc.tensor.matmul(
                        pc2[0:64, :], v_sb2[:, kt, 128 * m:128 * m + 64],
                        pra[:, j, :], start=(kp == 0 and j == 0),
                        stop=(kp == 7 and j == 1))
                    nc.tensor.matmul(
                        pc2[64:128, :],
                        v_sb2[:, kt, 128 * m + 64:128 * m + 128],
                        prb[:, j, :], start=(kp == 0 and j == 0),
                        stop=(kp == 7 and j == 1))
                if kp == 7:
                    rc = cap.tile([64, 2, 256], F32, name=f"crc{m}",
                                  tag="crc")
                    nc.vector.reciprocal_approx_fast(rc[:], psden[0:64, :, :])
                    nc.vector.tensor_mul(cctx_sb[0:64, m, :], pc2[0:64, :],
                                         rc[:, 0, :])
                    nc.vector.tensor_mul(cctx_sb[64:128, m, :],
                                         pc2[64:128, :], rc[:, 1, :])
            # cdense + residual -> h2, rmsnorm -> h2n -> AG
            d4 = pD.enter_context(tc.tile_pool(name="d4", bufs=1))
            h2_sb = d4.tile([128, 16, 256], F32)
            h2n_sb = d4.tile([128, 16, 256], BF16)
            for mt in range(16):
                ps = dps.tile([128, 256], F32, name="cdp", tag="dp")
                for kc in range(8):
                    nc.tensor.matmul(ps[:],
                                     wcd_sb[:, kc, mt * 128:mt * 128 + 128],
                                     cctx_sb[:, kc, :],
                                     start=(kc == 0), stop=(kc == 7))
                nc.vector.tensor_add(h2_sb[:, mt, :], ps[:],
                                     h1_sb[:, mt, :])
            pss2 = dps.tile([128, 256], F32, name="psd2", tag="dp")
            for kc in range(16):
                sq = scrp.tile([128, 256], BF16, name="sqd2", tag="sqc")
                nc.scalar.activation(sq[:], h2_sb[:, kc, :], SQ)
                nc.tensor.matmul(pss2[:], ones_bf[:], sq[:],
                                 start=(kc == 0), stop=(kc == 15))
            rms2 = scrp.tile([128, 256], F32, name="rmsd2", tag="rmsc")
            nc.scalar.activation(rms2[:], pss2[:], SQRT,
                                 scale=1.0 / H, bias=eps_sb[:])
            rinv2 = scrp.tile([128, 256], F32, name="rinvd", tag="rmsc")
            nc.vector.reciprocal_approx_fast(rinv2[:], rms2[:])
            for kc in range(16):
                nc.vector.tensor_mul(h2n_sb[:, kc, :],
                                     h2_sb[:, kc, :], rinv2[:])
            nc.sync.dma_start(r128(h2nb[:]), h2n_sb[:])
            nc.sync.dma_start(r128(h2out.ap()), h2_sb[:])
        pCD.close()
        nc.gpsimd.collective_compute(
            "AllGather", mybir.AluOpType.bypass, replica_groups=RG,
            ins=[h2nb.opt()], outs=[h2na.opt()])

        # ===== phase F: MLP (routed by expert ranges, bf16) =====
        with ExitStack() as pF:
            fp = pF.enter_context(tc.tile_pool(name="fp", bufs=1))
            hn_sb = fp.tile([128, 16, S], BF16)
            for r in range(NC_):
                nc.sync.dma_start(hn_sb[:, :, r * 256:r * 256 + 256],
                                  r128(h2na[r * H:(r + 1) * H, :]))
            fw = pF.enter_context(tc.tile_pool(name="fw", bufs=1))
            fps = pF.enter_context(tc.tile_pool(name="fps", bufs=2,
                                                space="PSUM"))
            fpd = pF.enter_context(tc.tile_pool(name="fpd", bufs=2,
                                                space="PSUM"))
            fac = pF.enter_context(tc.tile_pool(name="fac", bufs=2))
            fout = pF.enter_context(tc.tile_pool(name="fout", bufs=4))
            for ex, (lo, hi) in ((0, (0, b1)), (1, (b1, S))):
                gsrc = (wgu0, wgu1)[ex]
                dsrc = (wdn0, wdn1)[ex]
                dn_t = fw.tile([128, 6, H], BF16, name=f"dn{ex}", tag="dn")
                nc.sync.dma_start(dn_t[:], r128(dsrc.ap()))
                gwts = []
                for pi in range(6):
                    gw = 128 if pi < 5 else 48
                    gwt = fw.tile([128, 16, 256], BF16,
                                  name=f"guw{ex}{pi}", tag=f"guw{pi}")
                    nc.sync.dma_start(
                        gwt[:, :, :2 * gw],
                        r128(gsrc.ap()[:, pi * 256:pi * 256 + 2 * gw]))
                    gwts.append(gwt)
                for a0 in range(0, S, 512):
                    c0, c1 = max(a0, lo), min(a0 + 512, hi)
                    if c0 >= c1:
                        continue
                    t0_, W = a0, 512
                    eo, ew = c0 - a0, c1 - c0
                    act = fac.tile([128, 6, 512], BF16, name="act", tag="act")
                    for pi in range(6):
                        gw = 128 if pi < 5 else 48
                        gwt = gwts[pi]
                        pg = fps.tile([128, 512], F32, name="pg", tag="pg")
                        pu = fps.tile([128, 512], F32, name="pu", tag="pu")
                        for kc in range(16):
                            nc.tensor.matmul(pg[:gw, :W], gwt[:, kc, :gw],
                                             hn_sb[:, kc, t0_:t0_ + 512],
                                             start=(kc == 0), stop=(kc == 15))
                            nc.tensor.matmul(pu[:gw, :W], gwt[:, kc, gw:2 * gw],
                                             hn_sb[:, kc, t0_:t0_ + 512],
                                             start=(kc == 0), stop=(kc == 15))
                        gs = scrp.tile([128, 512], F32, name="gs", tag="gs")
                        nc.scalar.activation(gs[:gw, :W], pg[:gw, :W], SILU)
                        nc.vector.tensor_mul(act[:gw, pi, :W],
                                             gs[:gw, :W], pu[:gw, :W])
                    for mt in range(16):
                        pd = fpd.tile([128, 512], F32, name="pd", tag="pd")
                        for pi in range(6):
                            kw = 128 if pi < 5 else 48
                            nc.tensor.matmul(
                                pd[:, :W],
                                dn_t[:kw, pi, mt * 128:mt * 128 + 128],
                                act[:kw, pi, :W],
                                start=(pi == 0), stop=(pi == 5))
                        ot = fout.tile([128, 512], F32, name="fot", tag="fot")
                        if mt % 2 == 0:
                            nc.vector.tensor_copy(ot[:, eo:eo + ew],
                                                  pd[:, eo:eo + ew])
                        else:
                            nc.scalar.copy(ot[:, eo:eo + ew],
                                           pd[:, eo:eo + ew])
                        nc.sync.dma_start(
                            y.ap()[mt * 128:mt * 128 + 128, c0:c1],
                            ot[:, eo:eo + ew])
    nc.compile()
    return nc


_CACHE = {}


def kernel(**inputs):
    import ml_dtypes
    vm = np.asarray(inputs["vision_token_ids"]).astype(bool)
    lm = np.asarray(inputs["language_token_ids"]).astype(bool)
    g0 = np.where(vm & ~lm)[0]; g1 = np.where(vm & lm)[0]
    g2 = np.where(~vm & lm)[0]; g3 = np.where(~vm & ~lm)[0]
    perm = np.concatenate([g0, g1, g2, g3])
    b0 = len(g0); b1 = b0 + len(g1); b2 = b1 + len(g2)

    f32 = lambda x: np.ascontiguousarray(np.asarray(x, np.float32))
    bf = lambda x: np.ascontiguousarray(np.asarray(x).astype(ml_dtypes.bfloat16))
    pos = np.asarray(inputs["positions"]).astype(np.float32)
    half = HD // 2
    inv_freq = 1.0 / (ROPE_BASE ** (np.arange(half, dtype=np.float32) / half))
    fr = pos[:, None] * inv_freq[None, :]
    cos2 = np.concatenate([np.cos(fr)] * 2, 1).T[:, perm]
    sin2 = np.concatenate([np.sin(fr)] * 2, 1).T[:, perm]
    rot = np.zeros((HD, HD), np.float32)
    rot[np.arange(half), np.arange(half) + half] = -1.0
    rot[np.arange(half) + half, np.arange(half)] = 1.0
    op = np.asarray(inputs["positions"])[perm]

    # causal block states + diagonal multiplicative masks
    blk = []
    mrows = []
    for ci in range(4):
        qv = op[512 * ci:512 * ci + 512]
        for kt in range(16):
            kv = op[128 * kt:128 * kt + 128]
            if kv.max() <= qv.min():
                blk.append(1)
            elif kv.min() > qv.max():
                blk.append(0)
            else:
                blk.append(2)
                mrows.append((qv[None, :] >= kv[:, None]).astype(np.float32))
    blk = tuple(blk)
    maskd = (np.concatenate(mrows, 0) if mrows
             else np.zeros((128, 512), np.float32))

    # per-chunk expert combos (0=vis, 1=both, 2=lang, 3=neither)
    combo = np.full(S, 3, np.int8)
    combo[:b0] = 0; combo[b0:b1] = 1; combo[b1:b2] = 2
    chunk_combo = []
    uniform = True
    for j in range(NC_):
        cj = combo[256 * j:256 * j + 256]
        if (cj == cj[0]).all():
            chunk_combo.append(int(cj[0]))
        else:
            chunk_combo.append(-1)
            uniform = False

    wln_in = f32(inputs["w_ln_in"])[:, None]
    wln_pa = f32(inputs["w_ln_post_attn"])[:, None]
    wln_pc = f32(inputs["w_ln_post_cross"])[:, None]
    wqkv = [f32(inputs["w_vis_qkv"]) * wln_in, f32(inputs["w_lang_qkv"]) * wln_in]
    wd = [f32(inputs["w_vis_dense"]), f32(inputs["w_lang_dense"])]
    wgu = [f32(inputs["w_vis_gate_up"]) * wln_pc,
           f32(inputs["w_lang_gate_up"]) * wln_pc]
    wdn = [f32(inputs["w_vis_down"]), f32(inputs["w_lang_down"])]
    wkvf = f32(inputs["w_cross_kv"])
    hTp = f32(inputs["hidden_states"]).T[:, perm].copy()

    def interleave(w):  # w [H, 2*IS] = [gate | up]
        cols = []
        for i in range(5):
            cols.append(w[:, 128 * i:128 * i + 128])
            cols.append(w[:, IS + 128 * i:IS + 128 * i + 128])
        cols.append(w[:, 640:IS]); cols.append(w[:, IS + 640:2 * IS])
        return np.ascontiguousarray(np.concatenate(cols, 1))

    key = (b0, b1, b2, blk, uniform)
    if key not in _CACHE:
        _CACHE.clear()
        _CACHE[key] = build_kernel(b0, b1, b2, blk, uniform)
    nc = _CACHE[key]

    # dense weight combos (bf16, built once per distinct combo)
    wde_by_combo = {}
    for cb in set(chunk_combo):
        if cb == 0:
            wde_by_combo[cb] = bf(wd[0])
        elif cb == 1:
            wde_by_combo[cb] = bf(wd[0] + wd[1])
        elif cb == 2:
            wde_by_combo[cb] = bf(wd[1])
        elif cb == 3:
            wde_by_combo[cb] = bf(np.zeros_like(wd[0]))

    in_maps = []
    for c in range(NC_):
        qs = slice(256 * c, 256 * c + 256)
        m = dict(
            hT=bf(hTp),
            resid=bf(hTp[:, qs]),
            wqkv0=bf(np.concatenate([wqkv[0][:, qs], wqkv[0][:, H:][:, qs],
                                     wqkv[0][:, 2 * H:][:, qs]], 1)),
            wqkv1=bf(np.concatenate([wqkv[1][:, qs], wqkv[1][:, H:][:, qs],
                                     wqkv[1][:, 2 * H:][:, qs]], 1)),
            cos2=bf(cos2), sin2=bf(sin2), rotT=bf(rot.T),
            onesb=np.ones((128, 128), ml_dtypes.bfloat16),
            maskd=bf(maskd),
            encT=bf(f32(inputs["encoder_embeds"]).T),
            wkc=bf(wkvf[:, 128 * c:128 * c + 128]),
            wvc=bf(wkvf[:, CC + 128 * c:CC + 128 * c + 128]),
            wcq=bf(f32(inputs["w_cross_q"]) * wln_pa),
            wcd=bf(f32(inputs["w_cross_dense"])),
            wgu0=bf(interleave(np.concatenate(
                [wgu[0][:, IS * c:IS * c + IS],
                 wgu[0][:, I + IS * c:I + IS * c + IS]], 1))),
            wgu1=bf(interleave(np.concatenate(
                [wgu[1][:, IS * c:IS * c + IS],
                 wgu[1][:, I + IS * c:I + IS * c + IS]], 1))),
            wdn0=bf(np.concatenate([wdn[0][IS * c:IS * c + IS],
                                    np.zeros((ISP - IS, H), np.float32)], 0)),
            wdn1=bf(np.concatenate([wdn[1][IS * c:IS * c + IS],
                                    np.zeros((ISP - IS, H), np.float32)], 0)),
        )
        if uniform:
            m["wde"] = wde_by_combo[chunk_combo[c]]
        else:
            m["wde0"] = bf(wd[0])
            m["wde1"] = bf(wd[1])
            pv = vm[perm][qs].astype(np.float32)
            pl = lm[perm][qs].astype(np.float32)
            m["mv"] = bf(np.broadcast_to(pv[None, :], (128, 256)).copy())
            m["ml"] = bf(np.broadcast_to(pl[None, :], (128, 256)).copy())
        in_maps.append(m)

    # wqkv slot layout check: slots are [q(2x128) | k(2x128) | v(2x128)]
    # per-core head pair -> columns 128c..128c+256 of each of q,k,v.

    if os.environ.get("KSIM"):
        from concourse.bass_interp import MultiCoreSim
        sim = MultiCoreSim(nc, num_cores=NC_)
        for c, cs in sim.cores.items():
            for name, val in in_maps[c].items():
                cs.tensor(name)[:] = val
        sim.simulate(check_with_hw=False)
        results = [dict(y=np.array(sim.cores[c].tensor("y")),
                        h2out=np.array(sim.cores[c].tensor("h2out")))
                   for c in range(NC_)]
        kernel.last_exec_ns = 0
    else:
        trace = bool(int(os.environ.get("KTRACE", "0")))
        res = run_bass_kernel_spmd(nc, in_maps, core_ids=list(range(NC_)),
                                   trace=trace)
        kernel.last_exec_ns = res.exec_time_ns
        results = res.results
    tot = results[0]["y"].astype(np.float64)
    for c in range(1, NC_):
        tot += results[c]["y"]
    for c in range(NC_):
        tot[:, 256 * c:256 * c + 256] += results[c]["h2out"]
    out = np.empty((S, H), np.float32)
    out[perm, :] = tot.T.astype(np.float32)
    return out
